# revision 1
# baseline (speedup 1.0000x reference)
"""Trainium2 Bass kernel for nn_CNNModel_76312978915482.

Computation (bit-exact to the CPU-jax f32 reference):
  conv  = 2x2 all-ones conv, stride 2, pad 1 on x [B,1,330,314] -> [B,1,166,158]
          summed as (x00+x01)+(x10+x11)  (XLA CPU order, verified bit-exact)
  m     = min(conv, 0) min-pooled 2x2      ( == -maxpool(|min(conv,0)|), exact)
  s     = conv sum-pooled 2x2, summed ((c00+c01)+c10)+c11 (XLA CPU order)
  cond  = (m < lb) & ((s/4)/m > q1/lb)
  out[r,c] = 1.0 - cond[(r+1)//4 clip, (c+1)//4 clip]   (structured scatter)

The division-compare is evaluated as a product compare: for m < 0,
(s/4)/m > thr  <=>  s/4 < thr*m (reals)  and since fl scaling by 4 is exact,
NOT cond2 = (s >= fl(4thr * m)). One 0.5-ulp rounding against a verified
5.4e-6 (~45 ulp) minimum data-to-threshold gap: 0/1678592 flips vs the IEEE
divide reference on the actual dataset (validated on HW and host).

Layout: pure data parallel, batch 256 -> 32 images per core x 8 cores.
The host zero-pads each image to [332, 316]; a padded image is then exactly
83 contiguous blocks of 4*316 floats (block I = padded rows 4I..4I+3 =
original rows 4I-1..4I+2, one pooled row). Per core that gives a single
uniform stream of 32*83 = 2656 blocks. Jobs are tiled 128 partitions x
JPP=4 jobs per partition -> 5 full tiles (one dense contiguous 2.6 MB DMA
each way per tile) + one 96-job tail tile. Loads ride the SP HWDGE ring,
stores the Activation HWDGE ring; elementwise math on DVE; the 4x upsample
(step-0 broadcast copies) on GpSimd/Pool.
"""
import numpy as np

B, H, W = 256, 330, 314
Hp, Wp = 83, 79
NCORES = 8
BC = B // NCORES          # images per core (32)
H2, W2 = H + 2, W + 2     # padded image (332, 316)
BLK = 4 * W2              # floats per job block (1264)
HJ = W2 // 2              # conv cols (158)
NJOB = BC * Hp            # jobs per core (2656)
JPP = 4                   # max jobs per partition per tile
# (jobs_per_partition, partitions) per tile; small head tiles fill the
# pipeline quickly, small tail drains it quickly. Sum(jpp*P) == NJOB.
TILES = [(1, 128), (2, 128), (4, 128), (4, 128), (4, 128), (3, 128), (2, 128), (1, 96)]
assert sum(q * p for q, p in TILES) == NJOB
NSLOT = sum(q for q, _ in TILES)     # lb/thr table slots (21)

_CACHE: dict = {}


def _job_slot_table(v):
    """v[Hp, Wp] -> [128, NSLOT*Wp]: per tile t and local slot q, the column
    block on partition p holds v[job % Hp] for job = base_t + q*P_t + p."""
    tbl = np.zeros((128, NSLOT * Wp), np.float32)
    base = 0
    s = 0
    for q_n, P in TILES:
        for q in range(q_n):
            jobs = (base + q * P + np.arange(P)) % Hp
            tbl[:P, s * Wp:(s + 1) * Wp] = v[jobs]
            s += 1
        base += q_n * P
    return tbl


def _build_nc():
    import concourse.bacc as bacc
    import concourse.mybir as mybir
    import concourse.tile as tile

    dt = mybir.dt.float32
    A = mybir.AluOpType

    nc = bacc.Bacc("TRN2", target_bir_lowering=False, debug=False)
    xp_d = nc.dram_tensor("xp", [BC * H2 * W2], dt, kind="ExternalInput")
    lbx_d = nc.dram_tensor("lbx", [128, NSLOT * Wp], dt, kind="ExternalInput")
    thrx_d = nc.dram_tensor("thrx", [128, NSLOT * Wp], dt, kind="ExternalInput")
    out_d = nc.dram_tensor("out", [BC * H2 * W2], dt, kind="ExternalOutput")

    with tile.TileContext(nc) as tc:
        with tc.tile_pool(name="const", bufs=1) as cpool, \
             tc.tile_pool(name="bigx", bufs=3) as xpool, \
             tc.tile_pool(name="big", bufs=2) as bpool, \
             tc.tile_pool(name="small", bufs=2) as spool:
            lbt = cpool.tile([128, NSLOT * Wp], dt)
            thrt = cpool.tile([128, NSLOT * Wp], dt)
            # constants ride the (initially idle) Activation HWDGE ring
            nc.scalar.dma_start(lbt[:, :], lbx_d[:, :])
            nc.scalar.dma_start(thrt[:, :], thrx_d[:, :])

            def do_tile(j0, s0, P, jpp, last=False):
                """One tile: P partitions x jpp jobs each, jobs j0.., slots s0.."""
                nel = P * jpp * BLK
                ld_eng = nc.sync
                # late-tile stores ride the SP ring, which is idle once the
                # load stream finishes; earlier stores use the ACT ring
                st_eng = nc.sync if last else nc.scalar
                xt = xpool.tile([128, JPP * BLK], dt, tag="xt")
                xv = xt[:, :].rearrange("p (q r c) -> p q r c", q=JPP, r=4, c=W2)
                # dense contiguous load: job j -> (partition j%128, slot j//128)
                ld_eng.dma_start(
                    xt[:P, 0:jpp * BLK].rearrange(
                        "p (q f) -> p q f", q=jpp, f=BLK),
                    xp_d[j0 * BLK: j0 * BLK + nel].rearrange(
                        "(q p f) -> p q f", q=jpp, p=P, f=BLK))

                # hp[q, r, j] = x[q, r, 2j] + x[q, r, 2j+1]
                hp = bpool.tile([128, JPP * 4 * HJ], dt, tag="hp")
                hpv = hp[:, :].rearrange("p (q r j) -> p q r j", q=JPP, r=4, j=HJ)
                nc.vector.tensor_tensor(
                    hpv[:P, :jpp], xv[:P, :jpp, :, 0:W2:2],
                    xv[:P, :jpp, :, 1:W2:2], A.add)

                # conv rows: cv[q, i, j] = hp[q, 2i, j] + hp[q, 2i+1, j]
                cv = bpool.tile([128, JPP * 2 * HJ], dt, tag="cv")
                cvv = cv[:, :].rearrange("p (q i j) -> p q i j", q=JPP, i=2, j=HJ)
                nc.vector.tensor_tensor(
                    cvv[:P, :jpp], hpv[:P, :jpp, 0:4:2, :],
                    hpv[:P, :jpp, 1:4:2, :], A.add)

                c00 = cvv[:P, :jpp, 0, 0:HJ:2]
                c01 = cvv[:P, :jpp, 0, 1:HJ:2]
                c10 = cvv[:P, :jpp, 1, 0:HJ:2]
                c11 = cvv[:P, :jpp, 1, 1:HJ:2]

                def small(tag):
                    tl = spool.tile([128, JPP * Wp], dt, tag=tag)
                    return tl[:, :].rearrange("p (q j) -> p q j", q=JPP)[:P, :jpp]

                # m = min(c00, c01, c10, c11, 0)
                mn0 = small("mn0")
                nc.vector.scalar_tensor_tensor(mn0, c00, 0.0, c01, A.min, A.min)
                mn1 = small("mn1")
                nc.vector.scalar_tensor_tensor(mn1, c10, 0.0, c11, A.min, A.min)
                mv = small("mv")
                nc.vector.tensor_tensor(mv, mn0, mn1, A.min)

                # s = ((c00+c01)+c10)+c11   (XLA CPU reduce_window order)
                ut = small("ut")
                nc.vector.tensor_tensor(ut, c00, c01, A.add)
                s1 = small("s1")
                nc.vector.tensor_tensor(s1, ut, c10, A.add)
                sv = small("sv")
                nc.vector.tensor_tensor(sv, s1, c11, A.add)

                # o = 1 - (m<lb)&((s/4)/m>thr) = max(m>=lb, s>=fl(4thr*m))
                # (product compare; thrt holds 4*thr)
                sl = slice(s0 * Wp, (s0 + jpp) * Wp)
                lbv = lbt[:P, sl].rearrange("p (q j) -> p q j", q=jpp)
                thrv = thrt[:P, sl].rearrange("p (q j) -> p q j", q=jpp)
                tm = small("tm")
                nc.vector.tensor_tensor(tm, mv, thrv, A.mult)
                nc1 = small("nc1")
                nc.vector.tensor_tensor(nc1, mv, lbv, A.is_ge)
                nc2 = small("nc2")
                nc.vector.tensor_tensor(nc2, sv, tm, A.is_ge)
                ov = small("ov")
                nc.vector.tensor_tensor(ov, nc1, nc2, A.max)

                # expansion: ob[q, r, c'] = o[q, c'//4]
                ob = bpool.tile([128, JPP * BLK], dt, tag="ob")
                obv = ob[:, :].rearrange("p (q r c) -> p q r c", q=JPP, r=4, c=W2)
                nc.gpsimd.tensor_copy(
                    obv[:P, :jpp, 0, :].rearrange("p q (j k) -> p q j k", j=Wp, k=4),
                    ov.broadcast_to([P, jpp, Wp, 4]))
                nc.gpsimd.tensor_copy(
                    obv[:P, :jpp, 1:4, :],
                    obv[:P, :jpp, 0, :].unsqueeze(2).broadcast_to([P, jpp, 3, W2]))

                # dense contiguous store on the other HWDGE ring
                st_eng.dma_start(
                    out_d[j0 * BLK: j0 * BLK + nel].rearrange(
                        "(q p f) -> p q f", q=jpp, p=P, f=BLK),
                    ob[:P, 0:jpp * BLK].rearrange("p (q f) -> p q f", q=jpp, f=BLK))

            j0 = 0
            s0 = 0
            for ti, (q_n, P) in enumerate(TILES):
                do_tile(j0, s0, P, q_n, last=ti >= len(TILES) - 2)
                j0 += q_n * P
                s0 += q_n

    nc.compile()
    return nc


def get_nc():
    if "nc" not in _CACHE:
        _CACHE["nc"] = _build_nc()
    return _CACHE["nc"]


def _check_maps(map_rows, map_cols):
    """The device program hardcodes the clip(4i-1..4i+2) scatter footprint;
    verify the provided maps match it exactly."""
    off = np.arange(4)
    rows = np.clip(4 * np.arange(Hp)[:, None] - 1 + off[None, :], 0, H - 1)
    cols = np.clip(4 * np.arange(Wp)[:, None] - 1 + off[None, :], 0, W - 1)
    exp_rows = np.broadcast_to(rows[:, None, :, None], (Hp, Wp, 4, 4)).reshape(Hp, Wp, 16)
    exp_cols = np.broadcast_to(cols[None, :, None, :], (Hp, Wp, 4, 4)).reshape(Hp, Wp, 16)
    if not (np.asarray(map_rows) == exp_rows).all() or \
       not (np.asarray(map_cols) == exp_cols).all():
        raise ValueError("map_rows/map_cols do not match the expected "
                         "clip(4i-1..4i+2) footprint this kernel hardcodes")


def pad_input(x):
    """[n,1,H,W] (or [n,H,W]) f32 -> flat [n*H2*W2] with a zero ring per image."""
    if x.ndim == 4:
        x = x[:, 0]
    xp = np.zeros((x.shape[0], H2, W2), np.float32)
    xp[:, 1:H + 1, 1:W + 1] = x
    return np.ascontiguousarray(xp.reshape(-1))


def kernel(x, lower_bound1, q1, map_rows, map_cols):
    from concourse.bass_utils import run_bass_kernel_spmd

    x = np.asarray(x, dtype=np.float32)
    lb = np.ascontiguousarray(np.asarray(lower_bound1, dtype=np.float32))
    q1 = np.ascontiguousarray(np.asarray(q1, dtype=np.float32))
    _check_maps(map_rows, map_cols)
    assert x.shape == (B, 1, H, W), x.shape

    thr4 = (np.float32(4.0) * (q1 / lb).astype(np.float32)).astype(np.float32)
    lbx = _job_slot_table(lb)
    thrx = _job_slot_table(thr4)

    nc = get_nc()
    in_maps = [
        {"xp": pad_input(x[c * BC:(c + 1) * BC]), "lbx": lbx, "thrx": thrx}
        for c in range(NCORES)
    ]
    res = run_bass_kernel_spmd(nc, in_maps, list(range(NCORES)))
    out = np.concatenate(
        [r["out"].reshape(BC, H2, W2)[:, 1:H + 1, 1:W + 1] for r in res.results],
        axis=0)
    return np.ascontiguousarray(out.reshape(B, 1, H, W).astype(np.float32))



# revision 42
# speedup vs baseline: 2.2623x; 2.2623x over previous
"""Trainium2 Bass kernel for nn_CNNModel_76312978915482.

Computation (matches the CPU-jax f32 reference within the 2e-2 rel-err gate):
  conv  = 2x2 all-ones conv, stride 2, pad 1 on x [B,1,330,314] -> [B,1,166,158]
  m     = min over each 2x2 conv block            ( == -maxpool(|min(conv,0)|))
  s     = conv sum-pooled 2x2
  cond  = (m < lb) & ((s/4)/m > q1/lb)
  out[r,c] = 1.0 - cond[(r+1)//4 clip, (c+1)//4 clip]   (structured scatter)

This is a memory-regime problem, so the kernel minimizes HBM bytes:
  * x is loaded as fp16 (host converts; the data is N(0,1) so fp16 keeps
    11 mantissa bits). Measured on the actual dataset this flips 2208 of
    26.5M output pixels -> rel err 1.18e-2 vs the 2e-2 gate.
  * the 0/1 output is produced as int16 words 0x0101/0x0000 (one byte per
    pixel) and stored at 1 byte/pixel; the host reinterprets bytes as f32
    0/1. The division-compare is evaluated as a product compare
    (s >= 4*thr*m), thresholds are fp16 tables clipped to +-60000 so no
    inf/nan ever enters device arithmetic (tm is computed into f32).
Per core that is 6.71 MB in + 3.36 MB out + 0.85 MB tables ~ 10.9 MB vs
26.9 MB for the all-f32 version.

Layout: pure data parallel, batch 256 -> 32 images per core x 8 cores.
The host zero-pads each fp16 image to [332, 316]; a padded image is then
exactly 83 contiguous blocks of 4*316 halves (block I = padded rows
4I..4I+3 = original rows 4I-1..4I+2, one pooled row). Per core that gives
2656 uniform jobs tiled 128 partitions x up to 4 jobs/partition.

Engine split per job (per-partition free elems, cost-model effective):
  DVE : vertical add (fp16 packed, 2x), horizontal conv add, sum2 (2x),
        min2 (2x), s, m                                  ~ 1.0us/job-slot
  Pool: condition chain tm, nc1, nc2, or, x257 -> i16    ~ 0.6us/job-slot
  Act : one broadcast copy expands ov16[79] -> [4,79x2] output block,
        plus the store DMA ring;  SP: load DMA ring.
The int16 0x0101 trick makes the 4x column expansion a single x257
multiply: the i16 word *is* two identical output bytes, and the 4 rows of
an output block are identical i16 rows (one broadcast activation copy).
"""
import numpy as np

B, H, W = 256, 330, 314
Hp, Wp = 83, 79
NCORES = 8
BC = B // NCORES          # images per core (32)
H2, W2 = H + 2, W + 2     # padded image (332, 316)
BLK = 4 * W2              # fp16 elems per job block (1264)
OBLK = BLK // 2           # i16 elems per output job block (632)
HJ = W2 // 2              # conv cols (158)
NJOB = BC * Hp            # jobs per core (2656)
JPP = 4                   # max jobs per partition per tile
# (jobs_per_partition, partitions) per tile; small head tiles fill the
# pipeline quickly, small tail drains it quickly. Sum(jpp*P) == NJOB.
TILES = [(2, 128)] * 10 + [(1, 96)]
assert sum(q * p for q, p in TILES) == NJOB
NSLOT = sum(q for q, _ in TILES)     # lb/thr table slots (21)
XBUFS, BBUFS, SBUFS, OBUFS = 5, 3, 7, 5   # tile-pool depths
NTAIL = 1      # last NTAIL emitted tiles: conds on DVE, short drain chain
TAILEXP = "dve"  # engine for the tail tiles' expansion: act | pool | dve
NHOIST = 3     # compute the last NHOIST job-tiles FIRST (their stores then
               # fill DMA gaps and the job-stream end has no compute drain)
ORDER_OVERRIDE = None   # explicit emission order (list of tile indices)
TBL_RING = "sp"         # ring for the threshold-table DMAs: act | sp
TBL_POS = 3             # table DMAs queue behind this many tile loads
# per-op engine assignment for steady-state tiles (dve | pool).  The real
# Pool engine only accepts add/mult/tensor-scalar/copy TensorTensor forms
# (neuronxcc NCC_IXCG966 rejects min/max/is_ge on Pool), so the min/compare
# chain is pinned to DVE and Pool takes part of the add/mult work.
ENG = {"v": "dve", "c": "dve", "s2": "pool", "sv": "pool", "tm": "pool"}
EXP1, EXP2 = "act", "act"   # engines for the two expansion stages (act|pool|dve)

_CACHE: dict = {}


def _job_slot_table(v, dtype=np.float16):
    """v[Hp, Wp] -> [128, NSLOT*Wp]: per tile t and local slot q, the column
    block on partition p holds v[job % Hp] for job = base_t + q*P_t + p."""
    tbl = np.zeros((128, NSLOT * Wp), dtype)
    base = 0
    s = 0
    for q_n, P in TILES:
        for q in range(q_n):
            jobs = (base + q * P + np.arange(P)) % Hp
            tbl[:P, s * Wp:(s + 1) * Wp] = v[jobs]
            s += 1
        base += q_n * P
    return tbl


def _build_nc():
    import concourse.bacc as bacc
    import concourse.mybir as mybir
    import concourse.tile as tile

    f16 = mybir.dt.float16
    f32 = mybir.dt.float32
    i16 = mybir.dt.int16
    A = mybir.AluOpType

    nc = bacc.Bacc("TRN2", target_bir_lowering=False, debug=False)
    xp_d = nc.dram_tensor("xp", [BC * H2 * W2], f16, kind="ExternalInput")
    lbx_d = nc.dram_tensor("lbx", [128, NSLOT * Wp], f16, kind="ExternalInput")
    thrx_d = nc.dram_tensor("thrx", [128, NSLOT * Wp], f16, kind="ExternalInput")
    out_d = nc.dram_tensor("out", [BC * H2 * W2 // 2], i16, kind="ExternalOutput")

    with tile.TileContext(nc) as tc:
        with tc.tile_pool(name="const", bufs=1) as cpool, \
             tc.tile_pool(name="bigx", bufs=XBUFS) as xpool, \
             tc.tile_pool(name="big", bufs=BBUFS) as bpool, \
             tc.tile_pool(name="small", bufs=SBUFS) as spool, \
             tc.tile_pool(name="outp", bufs=OBUFS) as opool, \
             tc.tile_pool(name="hoist", bufs=1) as hpool:
            lbt = cpool.tile([128, NSLOT * Wp], f16)
            thrt = cpool.tile([128, NSLOT * Wp], f16)

            def emit_load(j0, P, jpp, xt):
                nel = P * jpp * BLK
                # dense contiguous load: job j -> (partition j%128, slot j//128)
                nc.sync.dma_start(
                    xt[:P, 0:jpp * BLK].rearrange(
                        "p (q f) -> p q f", q=jpp, f=BLK),
                    xp_d[j0 * BLK: j0 * BLK + nel].rearrange(
                        "(q p f) -> p q f", q=jpp, p=P, f=BLK))

            def emit_tables():
                eng = nc.sync if TBL_RING == "sp" else nc.scalar
                eng.dma_start(lbt[:, :], lbx_d[:, :])
                eng.dma_start(thrt[:, :], thrx_d[:, :])

            def do_tile(j0, s0, P, jpp, xt=None, last=False, tail=False, sfx=""):
                """One tile: P partitions x jpp jobs each, jobs j0.., slots s0..
                sfx != "" -> hoisted tile: single dedicated buffers (bufs=1
                pool, allocation sized to jpp) that never gate the main
                pipeline's buffer rotation."""
                # late-tile stores ride the SP ring, which is idle once the
                # load stream finishes; earlier stores use the ACT ring
                st_eng = nc.sync if last else nc.scalar
                QA = jpp if sfx else JPP        # allocation width (jobs)
                pools = (hpool if sfx else xpool, hpool if sfx else bpool,
                         hpool if sfx else spool, hpool if sfx else opool)
                xpoolQ, bpoolQ, spoolQ, opoolQ = pools
                if xt is None:
                    xt = xpoolQ.tile([128, QA * BLK], f16, tag="xt" + sfx)
                    emit_load(j0, P, jpp, xt)
                xv = xt[:, :].rearrange("p (q r c) -> p q r c", q=QA, r=4, c=W2)

                def eng(op):
                    if tail:
                        return nc.vector
                    return nc.gpsimd if ENG[op] == "pool" else nc.vector

                # vertical add (fp16 packed both sides -> 2x DVE mode):
                # v[q, r2, c] = x[q, 2 r2, c] + x[q, 2 r2 + 1, c]
                vt = bpoolQ.tile([128, QA * 2 * W2], f16, tag="vt" + sfx)
                vv = vt[:, :].rearrange("p (q r c) -> p q r c", q=QA, r=2, c=W2)
                eng("v").tensor_tensor(
                    vv[:P, :jpp], xv[:P, :jpp, 0:4:2, :],
                    xv[:P, :jpp, 1:4:2, :], A.add)

                # horizontal conv add: c[q, i, j] = v[q, i, 2j] + v[q, i, 2j+1]
                ct = bpoolQ.tile([128, QA * 2 * HJ], f16, tag="ct" + sfx)
                cv = ct[:, :].rearrange("p (q i j) -> p q i j", q=QA, i=2, j=HJ)
                eng("c").tensor_tensor(
                    cv[:P, :jpp], vv[:P, :jpp, :, 0:W2:2],
                    vv[:P, :jpp, :, 1:W2:2], A.add)

                def small(tag, dt, n=Wp):
                    tl = spoolQ.tile([128, QA * n], dt, tag=tag + sfx)
                    return tl[:, :].rearrange("p (q j) -> p q j", q=QA)[:P, :jpp]

                # row-pair combine at conv-col resolution (158 wide, 2x)
                s2 = small("s2", f16, HJ)
                eng("s2").tensor_tensor(s2, cv[:P, :jpp, 0, :],
                                        cv[:P, :jpp, 1, :], A.add)
                mn2 = small("mn2", f16, HJ)
                nc.vector.tensor_tensor(mn2, cv[:P, :jpp, 0, :],
                                        cv[:P, :jpp, 1, :], A.min)

                # col-pair combine down to pooled cells (79 wide)
                sv = small("sv", f32)
                eng("sv").tensor_tensor(sv, s2[:, :, 0:HJ:2],
                                        s2[:, :, 1:HJ:2], A.add)
                mv = small("mv", f16)
                nc.vector.tensor_tensor(mv, mn2[:, :, 0:HJ:2],
                                        mn2[:, :, 1:HJ:2], A.min)

                # cond_not = (m >= lb) | (s >= 4 thr m) on the Pool engine;
                # thrt holds clip(4 q1/lb, +-60000) fp16, tm lands in f32
                # (finite).  0/1 -> i16 0x0101/0x0000: the word is two
                # identical output bytes, so the column expansion is free.
                sl = slice(s0 * Wp, (s0 + jpp) * Wp)
                lbv = lbt[:P, sl].rearrange("p (q j) -> p q j", q=jpp)
                thrv = thrt[:P, sl].rearrange("p (q j) -> p q j", q=jpp)
                tm = small("tm", f32)
                eng("tm").tensor_tensor(tm, mv, thrv, A.mult)
                nc1 = small("nc1", f16)
                nc.vector.tensor_tensor(nc1, mv, lbv, A.is_ge)
                nc2 = small("nc2", f16)
                nc.vector.tensor_tensor(nc2, sv, tm, A.is_ge)
                ov = small("ov", f16)
                nc.vector.tensor_tensor(ov, nc1, nc2, A.max)

                # broadcast multiply + row broadcast expand ov[q, j] (0/1) to
                # the output block [q, 4 rows, j, 2 halves] scaled by 257 into
                # i16 0x0101/0x0000 words.  Two ops, each <= 3 free dims (the
                # Activation ISA rejects higher-rank access patterns).
                ob = opoolQ.tile([128, QA * OBLK], i16, tag="ob" + sfx)
                obv = ob[:, :].rearrange("p (q r w) -> p q r w", q=QA, r=4, w=HJ)
                row0 = obv[:P, :jpp, 0, :].rearrange("p q (j k) -> p q j k",
                                                     j=Wp, k=2)
                ovb = ov.unsqueeze(3).broadcast_to([P, jpp, Wp, 2])
                rows = obv[:P, :jpp, 1:4, :]
                r0b = obv[:P, :jpp, 0, :].unsqueeze(2).broadcast_to([P, jpp, 3, HJ])
                e1 = TAILEXP if tail else EXP1
                e2 = TAILEXP if tail else EXP2
                if e1 == "act":
                    nc.scalar.mul(row0, ovb, 257.0)
                elif e1 == "pool":
                    nc.gpsimd.tensor_scalar(row0, ovb, 257.0, None, A.mult)
                else:
                    nc.vector.tensor_scalar(row0, ovb, 257.0, None, A.mult)
                if e2 == "act":
                    nc.scalar.copy(rows, r0b)
                elif e2 == "pool":
                    nc.gpsimd.tensor_copy(rows, r0b)
                else:
                    nc.vector.tensor_copy(rows, r0b)

                # dense contiguous store (1264B per job) on the other ring
                noel = P * jpp * OBLK
                st_eng.dma_start(
                    out_d[j0 * OBLK: j0 * OBLK + noel].rearrange(
                        "(q p f) -> p q f", q=jpp, p=P, f=OBLK),
                    ob[:P, 0:jpp * OBLK].rearrange("p (q f) -> p q f", q=jpp, f=OBLK))

            # job-space offsets per tile
            offs = []
            j0 = 0
            s0 = 0
            for q_n, P in TILES:
                offs.append((j0, s0, P, q_n))
                j0 += q_n * P
                s0 += q_n
            n = len(TILES)
            # emission order: t0, t1, then the last NHOIST job-tiles (their
            # stores are ready early and fill DMA gaps near the end), then
            # the remaining middle tiles in job order
            if ORDER_OVERRIDE is not None:
                order = list(ORDER_OVERRIDE)
            else:
                order = [0, 1] + list(range(n - 1, n - 1 - NHOIST, -1)) + \
                    list(range(2, n - NHOIST))
            assert sorted(order) == list(range(n))
            # pre-emit the first TBL_POS tiles' loads so the table DMAs queue
            # behind them on the ring, then emit tables BEFORE any compute
            # that reads them (no use-before-def)
            pre = {}
            for ei in range(min(TBL_POS, n)):
                ti = order[ei]
                j0, s0, P, q_n = offs[ti]
                hoisted = ti >= n - NHOIST
                sfx = f"_h{ti}" if hoisted else ""
                QA = q_n if sfx else JPP
                xpoolQ = hpool if sfx else xpool
                xt = xpoolQ.tile([128, QA * BLK], f16, tag="xt" + sfx)
                emit_load(j0, P, q_n, xt)
                pre[ti] = xt
            emit_tables()
            for ei, ti in enumerate(order):
                j0, s0, P, q_n = offs[ti]
                hoisted = ti >= n - NHOIST
                do_tile(j0, s0, P, q_n, xt=pre.get(ti),
                        last=ei >= n - 2,
                        tail=ei >= n - NTAIL,
                        sfx=f"_h{ti}" if hoisted else "")
                del j0, s0

    nc.compile()
    return nc


def get_nc():
    if "nc" not in _CACHE:
        _CACHE["nc"] = _build_nc()
    return _CACHE["nc"]


def _check_maps(map_rows, map_cols):
    """The device program hardcodes the clip(4i-1..4i+2) scatter footprint;
    verify the provided maps match it exactly."""
    off = np.arange(4)
    rows = np.clip(4 * np.arange(Hp)[:, None] - 1 + off[None, :], 0, H - 1)
    cols = np.clip(4 * np.arange(Wp)[:, None] - 1 + off[None, :], 0, W - 1)
    exp_rows = np.broadcast_to(rows[:, None, :, None], (Hp, Wp, 4, 4)).reshape(Hp, Wp, 16)
    exp_cols = np.broadcast_to(cols[None, :, None, :], (Hp, Wp, 4, 4)).reshape(Hp, Wp, 16)
    if not (np.asarray(map_rows) == exp_rows).all() or \
       not (np.asarray(map_cols) == exp_cols).all():
        raise ValueError("map_rows/map_cols do not match the expected "
                         "clip(4i-1..4i+2) footprint this kernel hardcodes")


def pad_input(x):
    """[n,1,H,W] (or [n,H,W]) f32 -> flat fp16 [n*H2*W2] with a zero ring."""
    if x.ndim == 4:
        x = x[:, 0]
    xp = np.zeros((x.shape[0], H2, W2), np.float16)
    xp[:, 1:H + 1, 1:W + 1] = x.astype(np.float16)
    return np.ascontiguousarray(xp.reshape(-1))


def kernel(x, lower_bound1, q1, map_rows, map_cols):
    from concourse.bass_utils import run_bass_kernel_spmd

    x = np.asarray(x, dtype=np.float32)
    lb = np.ascontiguousarray(np.asarray(lower_bound1, dtype=np.float32))
    q1 = np.ascontiguousarray(np.asarray(q1, dtype=np.float32))
    _check_maps(map_rows, map_cols)
    assert x.shape == (B, 1, H, W), x.shape

    thr4 = (np.float32(4.0) * (q1 / lb).astype(np.float32)).astype(np.float32)
    lbx = _job_slot_table(lb.astype(np.float16))
    thrx = _job_slot_table(np.clip(thr4, -60000.0, 60000.0).astype(np.float16))

    nc = get_nc()
    in_maps = [
        {"xp": pad_input(x[c * BC:(c + 1) * BC]), "lbx": lbx, "thrx": thrx}
        for c in range(NCORES)
    ]
    res = run_bass_kernel_spmd(nc, in_maps, list(range(NCORES)))
    out = np.concatenate(
        [r["out"].view(np.uint8).reshape(BC, H2, W2)[:, 1:H + 1, 1:W + 1]
         for r in res.results],
        axis=0)
    return np.ascontiguousarray(out.reshape(B, 1, H, W).astype(np.float32))


# revision 48
# speedup vs baseline: 2.3197x; 1.0254x over previous
"""Trainium2 Bass kernel for nn_CNNModel_76312978915482.

Computation (matches the CPU-jax f32 reference within the 2e-2 rel-err gate):
  conv  = 2x2 all-ones conv, stride 2, pad 1 on x [B,1,330,314] -> [B,1,166,158]
  m     = min over each 2x2 conv block            ( == -maxpool(|min(conv,0)|))
  s     = conv sum-pooled 2x2
  cond  = (m < lb) & ((s/4)/m > q1/lb)
  out[r,c] = 1.0 - cond[(r+1)//4 clip, (c+1)//4 clip]   (structured scatter)

This is a memory-regime problem, so the kernel minimizes HBM bytes:
  * x is loaded as fp16 (host converts; the data is N(0,1) so fp16 keeps
    11 mantissa bits). Measured on the actual dataset this flips 2208 of
    26.5M output pixels -> rel err 1.18e-2 vs the 2e-2 gate.
  * the 0/1 output is produced as int16 words 0x0101/0x0000 (one byte per
    pixel) and stored at 1 byte/pixel; the host reinterprets bytes as f32
    0/1. The division-compare is evaluated as a product compare
    (s >= 4*thr*m), thresholds are fp16 tables clipped to +-60000 so no
    inf/nan ever enters device arithmetic (tm is computed into f32).
Per core that is 6.71 MB in + 3.36 MB out + 0.85 MB tables ~ 10.9 MB vs
26.9 MB for the all-f32 version.

Layout: pure data parallel, batch 256 -> 32 images per core x 8 cores.
The host zero-pads each fp16 image to [332, 316]; a padded image is then
exactly 83 contiguous blocks of 4*316 halves (block I = padded rows
4I..4I+3 = original rows 4I-1..4I+2, one pooled row). Per core that gives
2656 uniform jobs tiled 128 partitions x up to 4 jobs/partition.

Engine split per job (per-partition free elems, cost-model effective):
  DVE : vertical add (fp16 packed, 2x), horizontal conv add, sum2 (2x),
        min2 (2x), s, m                                  ~ 1.0us/job-slot
  Pool: condition chain tm, nc1, nc2, or, x257 -> i16    ~ 0.6us/job-slot
  Act : one broadcast copy expands ov16[79] -> [4,79x2] output block,
        plus the store DMA ring;  SP: load DMA ring.
The int16 0x0101 trick makes the 4x column expansion a single x257
multiply: the i16 word *is* two identical output bytes, and the 4 rows of
an output block are identical i16 rows (one broadcast activation copy).
"""
import numpy as np

B, H, W = 256, 330, 314
Hp, Wp = 83, 79
NCORES = 8
BC = B // NCORES          # images per core (32)
H2, W2 = H + 2, W + 2     # padded image (332, 316)
BLK = 4 * W2              # fp16 elems per job block (1264)
OBLK = BLK // 2           # i16 elems per output job block (632)
HJ = W2 // 2              # conv cols (158)
NJOB = BC * Hp            # jobs per core (2656)
JPP = 4                   # max jobs per partition per tile
# (jobs_per_partition, partitions) per tile; small head tiles fill the
# pipeline quickly, small tail drains it quickly. Sum(jpp*P) == NJOB.
TILES = [(2, 128)] * 10 + [(1, 96)]
assert sum(q * p for q, p in TILES) == NJOB
NSLOT = sum(q for q, _ in TILES)     # lb/thr table slots (21)
XBUFS, BBUFS, SBUFS, OBUFS = 5, 3, 7, 5   # tile-pool depths
NTAIL = 2      # last NTAIL emitted tiles: conds on DVE, short drain chain
TAILEXP = "dve"  # engine for the tail tiles' expansion: act | pool | dve
NHOIST = 3     # compute the last NHOIST job-tiles FIRST (their stores then
               # fill DMA gaps and the job-stream end has no compute drain)
ORDER_OVERRIDE = None   # explicit emission order (list of tile indices)
TBL_RING = "sp"         # ring for the threshold-table DMAs: act | sp
TBL_POS = 3             # table DMAs queue behind this many tile loads
# per-op engine assignment for steady-state tiles (dve | pool).  The real
# Pool engine only accepts add/mult/tensor-scalar/copy TensorTensor forms
# (neuronxcc NCC_IXCG966 rejects min/max/is_ge on Pool), so the min/compare
# chain is pinned to DVE and Pool takes part of the add/mult work.
ENG = {"v": "dve", "c": "dve", "s2": "pool", "sv": "pool", "tm": "pool"}
EXP1, EXP2 = "act", "act"   # engines for the two expansion stages (act|pool|dve)
SVTM16 = True   # sv/tm in fp16: nc2 becomes a 2x packed compare (tm may
                # overflow to +-inf; with thr clipped that is compare-safe)
CSPLIT = 40     # conv-add columns [158-CSPLIT, 158) ride Pool instead of DVE

_CACHE: dict = {}


def _job_slot_table(v, dtype=np.float16):
    """v[Hp, Wp] -> [128, NSLOT*Wp]: per tile t and local slot q, the column
    block on partition p holds v[job % Hp] for job = base_t + q*P_t + p."""
    tbl = np.zeros((128, NSLOT * Wp), dtype)
    base = 0
    s = 0
    for q_n, P in TILES:
        for q in range(q_n):
            jobs = (base + q * P + np.arange(P)) % Hp
            tbl[:P, s * Wp:(s + 1) * Wp] = v[jobs]
            s += 1
        base += q_n * P
    return tbl


def _build_nc():
    import concourse.bacc as bacc
    import concourse.mybir as mybir
    import concourse.tile as tile

    f16 = mybir.dt.float16
    f32 = mybir.dt.float32
    i16 = mybir.dt.int16
    A = mybir.AluOpType

    nc = bacc.Bacc("TRN2", target_bir_lowering=False, debug=False)
    xp_d = nc.dram_tensor("xp", [BC * H2 * W2], f16, kind="ExternalInput")
    lbx_d = nc.dram_tensor("lbx", [128, NSLOT * Wp], f16, kind="ExternalInput")
    thrx_d = nc.dram_tensor("thrx", [128, NSLOT * Wp], f16, kind="ExternalInput")
    out_d = nc.dram_tensor("out", [BC * H2 * W2 // 2], i16, kind="ExternalOutput")

    with tile.TileContext(nc) as tc:
        with tc.tile_pool(name="const", bufs=1) as cpool, \
             tc.tile_pool(name="bigx", bufs=XBUFS) as xpool, \
             tc.tile_pool(name="big", bufs=BBUFS) as bpool, \
             tc.tile_pool(name="small", bufs=SBUFS) as spool, \
             tc.tile_pool(name="outp", bufs=OBUFS) as opool, \
             tc.tile_pool(name="hoist", bufs=1) as hpool:
            lbt = cpool.tile([128, NSLOT * Wp], f16)
            thrt = cpool.tile([128, NSLOT * Wp], f16)

            def emit_load(j0, P, jpp, xt):
                nel = P * jpp * BLK
                # dense contiguous load: job j -> (partition j%128, slot j//128)
                nc.sync.dma_start(
                    xt[:P, 0:jpp * BLK].rearrange(
                        "p (q f) -> p q f", q=jpp, f=BLK),
                    xp_d[j0 * BLK: j0 * BLK + nel].rearrange(
                        "(q p f) -> p q f", q=jpp, p=P, f=BLK))

            def emit_tables():
                eng = nc.sync if TBL_RING == "sp" else nc.scalar
                eng.dma_start(lbt[:, :], lbx_d[:, :])
                eng.dma_start(thrt[:, :], thrx_d[:, :])

            def do_tile(j0, s0, P, jpp, xt=None, last=False, tail=False, sfx=""):
                """One tile: P partitions x jpp jobs each, jobs j0.., slots s0..
                sfx != "" -> hoisted tile: single dedicated buffers (bufs=1
                pool, allocation sized to jpp) that never gate the main
                pipeline's buffer rotation."""
                # late-tile stores ride the SP ring, which is idle once the
                # load stream finishes; earlier stores use the ACT ring
                st_eng = nc.sync if last else nc.scalar
                QA = jpp if sfx else JPP        # allocation width (jobs)
                pools = (hpool if sfx else xpool, hpool if sfx else bpool,
                         hpool if sfx else spool, hpool if sfx else opool)
                xpoolQ, bpoolQ, spoolQ, opoolQ = pools
                if xt is None:
                    xt = xpoolQ.tile([128, QA * BLK], f16, tag="xt" + sfx)
                    emit_load(j0, P, jpp, xt)
                xv = xt[:, :].rearrange("p (q r c) -> p q r c", q=QA, r=4, c=W2)

                def eng(op):
                    if tail:
                        return nc.vector
                    return nc.gpsimd if ENG[op] == "pool" else nc.vector

                # vertical add (fp16 packed both sides -> 2x DVE mode):
                # v[q, r2, c] = x[q, 2 r2, c] + x[q, 2 r2 + 1, c]
                vt = bpoolQ.tile([128, QA * 2 * W2], f16, tag="vt" + sfx)
                vv = vt[:, :].rearrange("p (q r c) -> p q r c", q=QA, r=2, c=W2)
                eng("v").tensor_tensor(
                    vv[:P, :jpp], xv[:P, :jpp, 0:4:2, :],
                    xv[:P, :jpp, 1:4:2, :], A.add)

                # horizontal conv add: c[q, i, j] = v[q, i, 2j] + v[q, i, 2j+1]
                ct = bpoolQ.tile([128, QA * 2 * HJ], f16, tag="ct" + sfx)
                cv = ct[:, :].rearrange("p (q i j) -> p q i j", q=QA, i=2, j=HJ)
                ncs = CSPLIT if not tail else 0
                jd = HJ - ncs     # columns computed by the main engine
                eng("c").tensor_tensor(
                    cv[:P, :jpp, :, 0:jd], vv[:P, :jpp, :, 0:2 * jd:2],
                    vv[:P, :jpp, :, 1:2 * jd:2], A.add)
                if ncs:
                    nc.gpsimd.tensor_tensor(
                        cv[:P, :jpp, :, jd:HJ], vv[:P, :jpp, :, 2 * jd:W2:2],
                        vv[:P, :jpp, :, 2 * jd + 1:W2:2], A.add)

                def small(tag, dt, n=Wp):
                    tl = spoolQ.tile([128, QA * n], dt, tag=tag + sfx)
                    return tl[:, :].rearrange("p (q j) -> p q j", q=QA)[:P, :jpp]

                # row-pair combine at conv-col resolution (158 wide, 2x)
                s2 = small("s2", f16, HJ)
                eng("s2").tensor_tensor(s2, cv[:P, :jpp, 0, :],
                                        cv[:P, :jpp, 1, :], A.add)
                mn2 = small("mn2", f16, HJ)
                nc.vector.tensor_tensor(mn2, cv[:P, :jpp, 0, :],
                                        cv[:P, :jpp, 1, :], A.min)

                # col-pair combine down to pooled cells (79 wide)
                svt = f16 if SVTM16 else f32
                sv = small("sv", svt)
                eng("sv").tensor_tensor(sv, s2[:, :, 0:HJ:2],
                                        s2[:, :, 1:HJ:2], A.add)
                mv = small("mv", f16)
                nc.vector.tensor_tensor(mv, mn2[:, :, 0:HJ:2],
                                        mn2[:, :, 1:HJ:2], A.min)

                # cond_not = (m >= lb) | (s >= 4 thr m) on the Pool engine;
                # thrt holds clip(4 q1/lb, +-60000) fp16, tm lands in f32
                # (finite).  0/1 -> i16 0x0101/0x0000: the word is two
                # identical output bytes, so the column expansion is free.
                sl = slice(s0 * Wp, (s0 + jpp) * Wp)
                lbv = lbt[:P, sl].rearrange("p (q j) -> p q j", q=jpp)
                thrv = thrt[:P, sl].rearrange("p (q j) -> p q j", q=jpp)
                tm = small("tm", svt)
                eng("tm").tensor_tensor(tm, mv, thrv, A.mult)
                nc1 = small("nc1", f16)
                nc.vector.tensor_tensor(nc1, mv, lbv, A.is_ge)
                nc2 = small("nc2", f16)
                nc.vector.tensor_tensor(nc2, sv, tm, A.is_ge)
                ov = small("ov", f16)
                nc.vector.tensor_tensor(ov, nc1, nc2, A.max)

                # broadcast multiply + row broadcast expand ov[q, j] (0/1) to
                # the output block [q, 4 rows, j, 2 halves] scaled by 257 into
                # i16 0x0101/0x0000 words.  Two ops, each <= 3 free dims (the
                # Activation ISA rejects higher-rank access patterns).
                ob = opoolQ.tile([128, QA * OBLK], i16, tag="ob" + sfx)
                obv = ob[:, :].rearrange("p (q r w) -> p q r w", q=QA, r=4, w=HJ)
                row0 = obv[:P, :jpp, 0, :].rearrange("p q (j k) -> p q j k",
                                                     j=Wp, k=2)
                ovb = ov.unsqueeze(3).broadcast_to([P, jpp, Wp, 2])
                rows = obv[:P, :jpp, 1:4, :]
                r0b = obv[:P, :jpp, 0, :].unsqueeze(2).broadcast_to([P, jpp, 3, HJ])
                e1 = TAILEXP if tail else EXP1
                e2 = TAILEXP if tail else EXP2
                if e1 == "act":
                    nc.scalar.mul(row0, ovb, 257.0)
                elif e1 == "pool":
                    nc.gpsimd.tensor_scalar(row0, ovb, 257.0, None, A.mult)
                else:
                    nc.vector.tensor_scalar(row0, ovb, 257.0, None, A.mult)
                if e2 == "act":
                    nc.scalar.copy(rows, r0b)
                elif e2 == "pool":
                    nc.gpsimd.tensor_copy(rows, r0b)
                else:
                    nc.vector.tensor_copy(rows, r0b)

                # dense contiguous store (1264B per job) on the other ring
                noel = P * jpp * OBLK
                st_eng.dma_start(
                    out_d[j0 * OBLK: j0 * OBLK + noel].rearrange(
                        "(q p f) -> p q f", q=jpp, p=P, f=OBLK),
                    ob[:P, 0:jpp * OBLK].rearrange("p (q f) -> p q f", q=jpp, f=OBLK))

            # job-space offsets per tile
            offs = []
            j0 = 0
            s0 = 0
            for q_n, P in TILES:
                offs.append((j0, s0, P, q_n))
                j0 += q_n * P
                s0 += q_n
            n = len(TILES)
            # emission order: t0, t1, then the last NHOIST job-tiles (their
            # stores are ready early and fill DMA gaps near the end), then
            # the remaining middle tiles in job order
            if ORDER_OVERRIDE is not None:
                order = list(ORDER_OVERRIDE)
            else:
                order = [0, 1] + list(range(n - 1, n - 1 - NHOIST, -1)) + \
                    list(range(2, n - NHOIST))
            assert sorted(order) == list(range(n))
            # pre-emit the first TBL_POS tiles' loads so the table DMAs queue
            # behind them on the ring, then emit tables BEFORE any compute
            # that reads them (no use-before-def)
            pre = {}
            for ei in range(min(TBL_POS, n)):
                ti = order[ei]
                j0, s0, P, q_n = offs[ti]
                hoisted = ti >= n - NHOIST
                sfx = f"_h{ti}" if hoisted else ""
                QA = q_n if sfx else JPP
                xpoolQ = hpool if sfx else xpool
                xt = xpoolQ.tile([128, QA * BLK], f16, tag="xt" + sfx)
                emit_load(j0, P, q_n, xt)
                pre[ti] = xt
            emit_tables()
            for ei, ti in enumerate(order):
                j0, s0, P, q_n = offs[ti]
                hoisted = ti >= n - NHOIST
                do_tile(j0, s0, P, q_n, xt=pre.get(ti),
                        last=ei >= n - 2,
                        tail=ei >= n - NTAIL,
                        sfx=f"_h{ti}" if hoisted else "")
                del j0, s0

    nc.compile()
    return nc


def get_nc():
    if "nc" not in _CACHE:
        _CACHE["nc"] = _build_nc()
    return _CACHE["nc"]


def _check_maps(map_rows, map_cols):
    """The device program hardcodes the clip(4i-1..4i+2) scatter footprint;
    verify the provided maps match it exactly."""
    off = np.arange(4)
    rows = np.clip(4 * np.arange(Hp)[:, None] - 1 + off[None, :], 0, H - 1)
    cols = np.clip(4 * np.arange(Wp)[:, None] - 1 + off[None, :], 0, W - 1)
    exp_rows = np.broadcast_to(rows[:, None, :, None], (Hp, Wp, 4, 4)).reshape(Hp, Wp, 16)
    exp_cols = np.broadcast_to(cols[None, :, None, :], (Hp, Wp, 4, 4)).reshape(Hp, Wp, 16)
    if not (np.asarray(map_rows) == exp_rows).all() or \
       not (np.asarray(map_cols) == exp_cols).all():
        raise ValueError("map_rows/map_cols do not match the expected "
                         "clip(4i-1..4i+2) footprint this kernel hardcodes")


def pad_input(x):
    """[n,1,H,W] (or [n,H,W]) f32 -> flat fp16 [n*H2*W2] with a zero ring."""
    if x.ndim == 4:
        x = x[:, 0]
    xp = np.zeros((x.shape[0], H2, W2), np.float16)
    xp[:, 1:H + 1, 1:W + 1] = x.astype(np.float16)
    return np.ascontiguousarray(xp.reshape(-1))


def kernel(x, lower_bound1, q1, map_rows, map_cols):
    from concourse.bass_utils import run_bass_kernel_spmd

    x = np.asarray(x, dtype=np.float32)
    lb = np.ascontiguousarray(np.asarray(lower_bound1, dtype=np.float32))
    q1 = np.ascontiguousarray(np.asarray(q1, dtype=np.float32))
    _check_maps(map_rows, map_cols)
    assert x.shape == (B, 1, H, W), x.shape

    thr4 = (np.float32(4.0) * (q1 / lb).astype(np.float32)).astype(np.float32)
    lbx = _job_slot_table(lb.astype(np.float16))
    thrx = _job_slot_table(np.clip(thr4, -60000.0, 60000.0).astype(np.float16))

    nc = get_nc()
    in_maps = [
        {"xp": pad_input(x[c * BC:(c + 1) * BC]), "lbx": lbx, "thrx": thrx}
        for c in range(NCORES)
    ]
    res = run_bass_kernel_spmd(nc, in_maps, list(range(NCORES)))
    out = np.concatenate(
        [r["out"].view(np.uint8).reshape(BC, H2, W2)[:, 1:H + 1, 1:W + 1]
         for r in res.results],
        axis=0)
    return np.ascontiguousarray(out.reshape(B, 1, H, W).astype(np.float32))


# revision 50
# speedup vs baseline: 2.3236x; 1.0017x over previous
"""Trainium2 Bass kernel for nn_CNNModel_76312978915482.

Computation (matches the CPU-jax f32 reference within the 2e-2 rel-err gate):
  conv  = 2x2 all-ones conv, stride 2, pad 1 on x [B,1,330,314] -> [B,1,166,158]
  m     = min over each 2x2 conv block            ( == -maxpool(|min(conv,0)|))
  s     = conv sum-pooled 2x2
  cond  = (m < lb) & ((s/4)/m > q1/lb)
  out[r,c] = 1.0 - cond[(r+1)//4 clip, (c+1)//4 clip]   (structured scatter)

This is a memory-regime problem, so the kernel minimizes HBM bytes:
  * x is loaded as fp16 (host converts; the data is N(0,1) so fp16 keeps
    11 mantissa bits). With fp16 intermediates this flips 2332 of 26.5M
    output pixels on the actual dataset -> rel err 1.22e-2 vs the 2e-2
    gate (hardware-verified bit-identical to the numpy model of this
    pipeline).
  * the 0/1 output is produced as int16 words 0x0101/0x0000 (one byte per
    pixel) and stored at 1 byte/pixel; the host reinterprets bytes as f32
    0/1. The division-compare is evaluated as a product compare
    (s >= 4*thr*m); thresholds are fp16 tables clipped to +-60000, so tm
    can only overflow to +-inf with the correct sign (compare-safe, no
    nan is ever produced).
Per core that is 6.71 MB in + 3.36 MB out + 0.85 MB tables ~ 10.9 MB vs
26.9 MB for the all-f32 version: the single shared DMA device (360 GB/s
in the cost model) is busy 30.3 us, and everything else hides under it.

Layout: pure data parallel, batch 256 -> 32 images per core x 8 cores.
The host zero-pads each fp16 image to [332, 316]; a padded image is then
exactly 83 contiguous blocks of 4*316 halves (block I = padded rows
4I..4I+3 = original rows 4I-1..4I+2, one pooled row). Per core that gives
2656 uniform jobs tiled 128 partitions x up to 2 jobs/partition.

Engine split (the real Pool engine only accepts add/mult/scalar/copy ops,
so the min/compare chain must stay on DVE):
  DVE : vertical add (fp16 packed, 2x mode), most of the conv add, min2
        (2x), m, and the three compares
  Pool: sum2, s, 4*thr*m, and a 40-column slice of the conv add
  Act : two broadcast ops expand ov[79] (0/1) -> i16 0x0101 output block
        (x257 pair-broadcast + row broadcast), plus the store DMA ring
  SP  : load DMA ring (threshold tables queue behind the first 3 loads).
The int16 0x0101 trick makes the 4x column expansion a single x257
multiply: the i16 word *is* two identical output bytes, and the 4 rows of
an output block are identical i16 rows (one broadcast copy).
The last two job-tiles are computed FIRST (dedicated buffers) so their
stores fill DMA gaps at the end; the last emitted tiles run their whole
chain on DVE to drain without cross-engine hops.
"""
import numpy as np

B, H, W = 256, 330, 314
Hp, Wp = 83, 79
NCORES = 8
BC = B // NCORES          # images per core (32)
H2, W2 = H + 2, W + 2     # padded image (332, 316)
BLK = 4 * W2              # fp16 elems per job block (1264)
OBLK = BLK // 2           # i16 elems per output job block (632)
HJ = W2 // 2              # conv cols (158)
NJOB = BC * Hp            # jobs per core (2656)
JPP = 4                   # max jobs per partition per tile
# (jobs_per_partition, partitions) per tile; small head tiles fill the
# pipeline quickly, small tail drains it quickly. Sum(jpp*P) == NJOB.
TILES = [(1, 128), (1, 128)] + [(2, 128)] * 9 + [(1, 96)]
assert sum(q * p for q, p in TILES) == NJOB
NSLOT = sum(q for q, _ in TILES)     # lb/thr table slots (21)
XBUFS, BBUFS, SBUFS, OBUFS = 5, 3, 7, 5   # tile-pool depths
NTAIL = 2      # last NTAIL emitted tiles: conds on DVE, short drain chain
TAILEXP = "dve"  # engine for the tail tiles' expansion: act | pool | dve
NHOIST = 2     # compute the last NHOIST job-tiles FIRST (their stores then
               # fill DMA gaps and the job-stream end has no compute drain)
ORDER_OVERRIDE = None   # explicit emission order (list of tile indices)
TBL_RING = "sp"         # ring for the threshold-table DMAs: act | sp
TBL_POS = 3             # table DMAs queue behind this many tile loads
# per-op engine assignment for steady-state tiles (dve | pool).  The real
# Pool engine only accepts add/mult/tensor-scalar/copy TensorTensor forms
# (neuronxcc NCC_IXCG966 rejects min/max/is_ge on Pool), so the min/compare
# chain is pinned to DVE and Pool takes part of the add/mult work.
ENG = {"v": "dve", "c": "dve", "s2": "pool", "sv": "pool", "tm": "pool"}
EXP1, EXP2 = "act", "act"   # engines for the two expansion stages (act|pool|dve)
SVTM16 = True   # sv/tm in fp16: nc2 becomes a 2x packed compare (tm may
                # overflow to +-inf; with thr clipped that is compare-safe)
CSPLIT = 40     # conv-add columns [158-CSPLIT, 158) ride Pool instead of DVE

_CACHE: dict = {}


def _job_slot_table(v, dtype=np.float16):
    """v[Hp, Wp] -> [128, NSLOT*Wp]: per tile t and local slot q, the column
    block on partition p holds v[job % Hp] for job = base_t + q*P_t + p."""
    tbl = np.zeros((128, NSLOT * Wp), dtype)
    base = 0
    s = 0
    for q_n, P in TILES:
        for q in range(q_n):
            jobs = (base + q * P + np.arange(P)) % Hp
            tbl[:P, s * Wp:(s + 1) * Wp] = v[jobs]
            s += 1
        base += q_n * P
    return tbl


def _build_nc():
    import concourse.bacc as bacc
    import concourse.mybir as mybir
    import concourse.tile as tile

    f16 = mybir.dt.float16
    f32 = mybir.dt.float32
    i16 = mybir.dt.int16
    A = mybir.AluOpType

    nc = bacc.Bacc("TRN2", target_bir_lowering=False, debug=False)
    xp_d = nc.dram_tensor("xp", [BC * H2 * W2], f16, kind="ExternalInput")
    lbx_d = nc.dram_tensor("lbx", [128, NSLOT * Wp], f16, kind="ExternalInput")
    thrx_d = nc.dram_tensor("thrx", [128, NSLOT * Wp], f16, kind="ExternalInput")
    out_d = nc.dram_tensor("out", [BC * H2 * W2 // 2], i16, kind="ExternalOutput")

    with tile.TileContext(nc) as tc:
        with tc.tile_pool(name="const", bufs=1) as cpool, \
             tc.tile_pool(name="bigx", bufs=XBUFS) as xpool, \
             tc.tile_pool(name="big", bufs=BBUFS) as bpool, \
             tc.tile_pool(name="small", bufs=SBUFS) as spool, \
             tc.tile_pool(name="outp", bufs=OBUFS) as opool, \
             tc.tile_pool(name="hoist", bufs=1) as hpool:
            lbt = cpool.tile([128, NSLOT * Wp], f16)
            thrt = cpool.tile([128, NSLOT * Wp], f16)

            def emit_load(j0, P, jpp, xt):
                nel = P * jpp * BLK
                # dense contiguous load: job j -> (partition j%128, slot j//128)
                nc.sync.dma_start(
                    xt[:P, 0:jpp * BLK].rearrange(
                        "p (q f) -> p q f", q=jpp, f=BLK),
                    xp_d[j0 * BLK: j0 * BLK + nel].rearrange(
                        "(q p f) -> p q f", q=jpp, p=P, f=BLK))

            def emit_tables():
                eng = nc.sync if TBL_RING == "sp" else nc.scalar
                eng.dma_start(lbt[:, :], lbx_d[:, :])
                eng.dma_start(thrt[:, :], thrx_d[:, :])

            def do_tile(j0, s0, P, jpp, xt=None, last=False, tail=False, sfx=""):
                """One tile: P partitions x jpp jobs each, jobs j0.., slots s0..
                sfx != "" -> hoisted tile: single dedicated buffers (bufs=1
                pool, allocation sized to jpp) that never gate the main
                pipeline's buffer rotation."""
                # late-tile stores ride the SP ring, which is idle once the
                # load stream finishes; earlier stores use the ACT ring
                st_eng = nc.sync if last else nc.scalar
                QA = jpp if sfx else JPP        # allocation width (jobs)
                pools = (hpool if sfx else xpool, hpool if sfx else bpool,
                         hpool if sfx else spool, hpool if sfx else opool)
                xpoolQ, bpoolQ, spoolQ, opoolQ = pools
                if xt is None:
                    xt = xpoolQ.tile([128, QA * BLK], f16, tag="xt" + sfx)
                    emit_load(j0, P, jpp, xt)
                xv = xt[:, :].rearrange("p (q r c) -> p q r c", q=QA, r=4, c=W2)

                def eng(op):
                    if tail:
                        return nc.vector
                    return nc.gpsimd if ENG[op] == "pool" else nc.vector

                # vertical add (fp16 packed both sides -> 2x DVE mode):
                # v[q, r2, c] = x[q, 2 r2, c] + x[q, 2 r2 + 1, c]
                vt = bpoolQ.tile([128, QA * 2 * W2], f16, tag="vt" + sfx)
                vv = vt[:, :].rearrange("p (q r c) -> p q r c", q=QA, r=2, c=W2)
                eng("v").tensor_tensor(
                    vv[:P, :jpp], xv[:P, :jpp, 0:4:2, :],
                    xv[:P, :jpp, 1:4:2, :], A.add)

                # horizontal conv add: c[q, i, j] = v[q, i, 2j] + v[q, i, 2j+1]
                ct = bpoolQ.tile([128, QA * 2 * HJ], f16, tag="ct" + sfx)
                cv = ct[:, :].rearrange("p (q i j) -> p q i j", q=QA, i=2, j=HJ)
                ncs = CSPLIT if not tail else 0
                jd = HJ - ncs     # columns computed by the main engine
                eng("c").tensor_tensor(
                    cv[:P, :jpp, :, 0:jd], vv[:P, :jpp, :, 0:2 * jd:2],
                    vv[:P, :jpp, :, 1:2 * jd:2], A.add)
                if ncs:
                    nc.gpsimd.tensor_tensor(
                        cv[:P, :jpp, :, jd:HJ], vv[:P, :jpp, :, 2 * jd:W2:2],
                        vv[:P, :jpp, :, 2 * jd + 1:W2:2], A.add)

                def small(tag, dt, n=Wp):
                    tl = spoolQ.tile([128, QA * n], dt, tag=tag + sfx)
                    return tl[:, :].rearrange("p (q j) -> p q j", q=QA)[:P, :jpp]

                # row-pair combine at conv-col resolution (158 wide, 2x)
                s2 = small("s2", f16, HJ)
                eng("s2").tensor_tensor(s2, cv[:P, :jpp, 0, :],
                                        cv[:P, :jpp, 1, :], A.add)
                mn2 = small("mn2", f16, HJ)
                nc.vector.tensor_tensor(mn2, cv[:P, :jpp, 0, :],
                                        cv[:P, :jpp, 1, :], A.min)

                # col-pair combine down to pooled cells (79 wide)
                svt = f16 if SVTM16 else f32
                sv = small("sv", svt)
                eng("sv").tensor_tensor(sv, s2[:, :, 0:HJ:2],
                                        s2[:, :, 1:HJ:2], A.add)
                mv = small("mv", f16)
                nc.vector.tensor_tensor(mv, mn2[:, :, 0:HJ:2],
                                        mn2[:, :, 1:HJ:2], A.min)

                # cond_not = (m >= lb) | (s >= 4 thr m) on the Pool engine;
                # thrt holds clip(4 q1/lb, +-60000) fp16, tm lands in f32
                # (finite).  0/1 -> i16 0x0101/0x0000: the word is two
                # identical output bytes, so the column expansion is free.
                sl = slice(s0 * Wp, (s0 + jpp) * Wp)
                lbv = lbt[:P, sl].rearrange("p (q j) -> p q j", q=jpp)
                thrv = thrt[:P, sl].rearrange("p (q j) -> p q j", q=jpp)
                tm = small("tm", svt)
                eng("tm").tensor_tensor(tm, mv, thrv, A.mult)
                nc1 = small("nc1", f16)
                nc.vector.tensor_tensor(nc1, mv, lbv, A.is_ge)
                nc2 = small("nc2", f16)
                nc.vector.tensor_tensor(nc2, sv, tm, A.is_ge)
                ov = small("ov", f16)
                nc.vector.tensor_tensor(ov, nc1, nc2, A.max)

                # broadcast multiply + row broadcast expand ov[q, j] (0/1) to
                # the output block [q, 4 rows, j, 2 halves] scaled by 257 into
                # i16 0x0101/0x0000 words.  Two ops, each <= 3 free dims (the
                # Activation ISA rejects higher-rank access patterns).
                ob = opoolQ.tile([128, QA * OBLK], i16, tag="ob" + sfx)
                obv = ob[:, :].rearrange("p (q r w) -> p q r w", q=QA, r=4, w=HJ)
                row0 = obv[:P, :jpp, 0, :].rearrange("p q (j k) -> p q j k",
                                                     j=Wp, k=2)
                ovb = ov.unsqueeze(3).broadcast_to([P, jpp, Wp, 2])
                rows = obv[:P, :jpp, 1:4, :]
                r0b = obv[:P, :jpp, 0, :].unsqueeze(2).broadcast_to([P, jpp, 3, HJ])
                e1 = TAILEXP if tail else EXP1
                e2 = TAILEXP if tail else EXP2
                if e1 == "act":
                    nc.scalar.mul(row0, ovb, 257.0)
                elif e1 == "pool":
                    nc.gpsimd.tensor_scalar(row0, ovb, 257.0, None, A.mult)
                else:
                    nc.vector.tensor_scalar(row0, ovb, 257.0, None, A.mult)
                if e2 == "act":
                    nc.scalar.copy(rows, r0b)
                elif e2 == "pool":
                    nc.gpsimd.tensor_copy(rows, r0b)
                else:
                    nc.vector.tensor_copy(rows, r0b)

                # dense contiguous store (1264B per job) on the other ring
                noel = P * jpp * OBLK
                st_eng.dma_start(
                    out_d[j0 * OBLK: j0 * OBLK + noel].rearrange(
                        "(q p f) -> p q f", q=jpp, p=P, f=OBLK),
                    ob[:P, 0:jpp * OBLK].rearrange("p (q f) -> p q f", q=jpp, f=OBLK))

            # job-space offsets per tile
            offs = []
            j0 = 0
            s0 = 0
            for q_n, P in TILES:
                offs.append((j0, s0, P, q_n))
                j0 += q_n * P
                s0 += q_n
            n = len(TILES)
            # emission order: t0, t1, then the last NHOIST job-tiles (their
            # stores are ready early and fill DMA gaps near the end), then
            # the remaining middle tiles in job order
            if ORDER_OVERRIDE is not None:
                order = list(ORDER_OVERRIDE)
            else:
                order = [0, 1] + list(range(n - 1, n - 1 - NHOIST, -1)) + \
                    list(range(2, n - NHOIST))
            assert sorted(order) == list(range(n))
            # pre-emit the first TBL_POS tiles' loads so the table DMAs queue
            # behind them on the ring, then emit tables BEFORE any compute
            # that reads them (no use-before-def)
            pre = {}
            for ei in range(min(TBL_POS, n)):
                ti = order[ei]
                j0, s0, P, q_n = offs[ti]
                hoisted = ti >= n - NHOIST
                sfx = f"_h{ti}" if hoisted else ""
                QA = q_n if sfx else JPP
                xpoolQ = hpool if sfx else xpool
                xt = xpoolQ.tile([128, QA * BLK], f16, tag="xt" + sfx)
                emit_load(j0, P, q_n, xt)
                pre[ti] = xt
            emit_tables()
            for ei, ti in enumerate(order):
                j0, s0, P, q_n = offs[ti]
                hoisted = ti >= n - NHOIST
                do_tile(j0, s0, P, q_n, xt=pre.get(ti),
                        last=ei >= n - 2,
                        tail=ei >= n - NTAIL,
                        sfx=f"_h{ti}" if hoisted else "")
                del j0, s0

    nc.compile()
    return nc


def get_nc():
    if "nc" not in _CACHE:
        _CACHE["nc"] = _build_nc()
    return _CACHE["nc"]


def _check_maps(map_rows, map_cols):
    """The device program hardcodes the clip(4i-1..4i+2) scatter footprint;
    verify the provided maps match it exactly."""
    off = np.arange(4)
    rows = np.clip(4 * np.arange(Hp)[:, None] - 1 + off[None, :], 0, H - 1)
    cols = np.clip(4 * np.arange(Wp)[:, None] - 1 + off[None, :], 0, W - 1)
    exp_rows = np.broadcast_to(rows[:, None, :, None], (Hp, Wp, 4, 4)).reshape(Hp, Wp, 16)
    exp_cols = np.broadcast_to(cols[None, :, None, :], (Hp, Wp, 4, 4)).reshape(Hp, Wp, 16)
    if not (np.asarray(map_rows) == exp_rows).all() or \
       not (np.asarray(map_cols) == exp_cols).all():
        raise ValueError("map_rows/map_cols do not match the expected "
                         "clip(4i-1..4i+2) footprint this kernel hardcodes")


def pad_input(x):
    """[n,1,H,W] (or [n,H,W]) f32 -> flat fp16 [n*H2*W2] with a zero ring."""
    if x.ndim == 4:
        x = x[:, 0]
    xp = np.zeros((x.shape[0], H2, W2), np.float16)
    xp[:, 1:H + 1, 1:W + 1] = x.astype(np.float16)
    return np.ascontiguousarray(xp.reshape(-1))


def kernel(x, lower_bound1, q1, map_rows, map_cols):
    from concourse.bass_utils import run_bass_kernel_spmd

    x = np.asarray(x, dtype=np.float32)
    lb = np.ascontiguousarray(np.asarray(lower_bound1, dtype=np.float32))
    q1 = np.ascontiguousarray(np.asarray(q1, dtype=np.float32))
    _check_maps(map_rows, map_cols)
    assert x.shape == (B, 1, H, W), x.shape

    thr4 = (np.float32(4.0) * (q1 / lb).astype(np.float32)).astype(np.float32)
    lbx = _job_slot_table(lb.astype(np.float16))
    thrx = _job_slot_table(np.clip(thr4, -60000.0, 60000.0).astype(np.float16))

    nc = get_nc()
    in_maps = [
        {"xp": pad_input(x[c * BC:(c + 1) * BC]), "lbx": lbx, "thrx": thrx}
        for c in range(NCORES)
    ]
    res = run_bass_kernel_spmd(nc, in_maps, list(range(NCORES)))
    out = np.concatenate(
        [r["out"].view(np.uint8).reshape(BC, H2, W2)[:, 1:H + 1, 1:W + 1]
         for r in res.results],
        axis=0)
    return np.ascontiguousarray(out.reshape(B, 1, H, W).astype(np.float32))


# revision 52
# speedup vs baseline: 2.3431x; 1.0084x over previous
"""Trainium2 Bass kernel for nn_CNNModel_76312978915482.

Computation (matches the CPU-jax f32 reference within the 2e-2 rel-err gate):
  conv  = 2x2 all-ones conv, stride 2, pad 1 on x [B,1,330,314] -> [B,1,166,158]
  m     = min over each 2x2 conv block            ( == -maxpool(|min(conv,0)|))
  s     = conv sum-pooled 2x2
  cond  = (m < lb) & ((s/4)/m > q1/lb)
  out[r,c] = 1.0 - cond[(r+1)//4 clip, (c+1)//4 clip]   (structured scatter)

This is a memory-regime problem, so the kernel minimizes HBM bytes:
  * x is loaded as fp16 (host converts; the data is N(0,1) so fp16 keeps
    11 mantissa bits). With fp16 intermediates this flips 2332 of 26.5M
    output pixels on the actual dataset -> rel err 1.22e-2 vs the 2e-2
    gate (hardware-verified bit-identical to the numpy model of this
    pipeline).
  * the 0/1 output is produced as int16 words 0x0101/0x0000 (one byte per
    pixel) and stored at 1 byte/pixel; the host reinterprets bytes as f32
    0/1. The division-compare is evaluated as a product compare
    (s >= 4*thr*m); thresholds are fp16 tables clipped to +-60000, so tm
    can only overflow to +-inf with the correct sign (compare-safe, no
    nan is ever produced).
Per core that is 6.71 MB in + 3.36 MB out + 0.85 MB tables ~ 10.9 MB vs
26.9 MB for the all-f32 version: the single shared DMA device (360 GB/s
in the cost model) is busy 30.3 us, and everything else hides under it.

Layout: pure data parallel, batch 256 -> 32 images per core x 8 cores.
The host zero-pads each fp16 image to [332, 316]; a padded image is then
exactly 83 contiguous blocks of 4*316 halves (block I = padded rows
4I..4I+3 = original rows 4I-1..4I+2, one pooled row). Per core that gives
2656 uniform jobs tiled 128 partitions x up to 2 jobs/partition.

Engine split (the real Pool engine only accepts add/mult/scalar/copy ops,
so the min/compare chain must stay on DVE):
  DVE : vertical add (fp16 packed, 2x mode), most of the conv add, min2
        (2x), m, and the three compares
  Pool: sum2, s, 4*thr*m, and a 40-column slice of the conv add
  Act : two broadcast ops expand ov[79] (0/1) -> i16 0x0101 output block
        (x257 pair-broadcast + row broadcast), plus the store DMA ring
  SP  : load DMA ring (threshold tables queue behind the first 4 loads).
The int16 0x0101 trick makes the 4x column expansion a single x257
multiply: the i16 word *is* two identical output bytes, and the 4 rows of
an output block are identical i16 rows (one broadcast copy).
The last two job-tiles are computed FIRST (dedicated buffers) so their
stores fill DMA gaps at the end; the last emitted tiles run their whole
chain on DVE to drain without cross-engine hops.
"""
import numpy as np

B, H, W = 256, 330, 314
Hp, Wp = 83, 79
NCORES = 8
BC = B // NCORES          # images per core (32)
H2, W2 = H + 2, W + 2     # padded image (332, 316)
BLK = 4 * W2              # fp16 elems per job block (1264)
OBLK = BLK // 2           # i16 elems per output job block (632)
HJ = W2 // 2              # conv cols (158)
NJOB = BC * Hp            # jobs per core (2656)
JPP = 4                   # max jobs per partition per tile
# (jobs_per_partition, partitions) per tile; small head tiles fill the
# pipeline quickly, small tail drains it quickly. Sum(jpp*P) == NJOB.
TILES = [(1, 128), (1, 128)] + [(2, 128)] * 9 + [(1, 96)]
assert sum(q * p for q, p in TILES) == NJOB
NSLOT = sum(q for q, _ in TILES)     # lb/thr table slots (21)
XBUFS, BBUFS, SBUFS, OBUFS = 5, 3, 7, 5   # tile-pool depths
NTAIL = 1      # last NTAIL emitted tiles: conds on DVE, short drain chain
TAILEXP = "dve"  # engine for the tail tiles' expansion: act | pool | dve
NHOIST = 2     # compute the last NHOIST job-tiles FIRST (their stores then
               # fill DMA gaps and the job-stream end has no compute drain)
ORDER_OVERRIDE = None   # explicit emission order (list of tile indices)
TBL_RING = "sp"         # ring for the threshold-table DMAs: act | sp
TBL_POS = 4             # table DMAs queue behind this many tile loads
# per-op engine assignment for steady-state tiles (dve | pool).  The real
# Pool engine only accepts add/mult/tensor-scalar/copy TensorTensor forms
# (neuronxcc NCC_IXCG966 rejects min/max/is_ge on Pool), so the min/compare
# chain is pinned to DVE and Pool takes part of the add/mult work.
ENG = {"v": "dve", "c": "dve", "s2": "pool", "sv": "pool", "tm": "pool"}
EXP1, EXP2 = "act", "act"   # engines for the two expansion stages (act|pool|dve)
SVTM16 = True   # sv/tm in fp16: nc2 becomes a 2x packed compare (tm may
                # overflow to +-inf; with thr clipped that is compare-safe)
CSPLIT = 40     # conv-add columns [158-CSPLIT, 158) ride Pool instead of DVE

_CACHE: dict = {}


def _job_slot_table(v, dtype=np.float16):
    """v[Hp, Wp] -> [128, NSLOT*Wp]: per tile t and local slot q, the column
    block on partition p holds v[job % Hp] for job = base_t + q*P_t + p."""
    tbl = np.zeros((128, NSLOT * Wp), dtype)
    base = 0
    s = 0
    for q_n, P in TILES:
        for q in range(q_n):
            jobs = (base + q * P + np.arange(P)) % Hp
            tbl[:P, s * Wp:(s + 1) * Wp] = v[jobs]
            s += 1
        base += q_n * P
    return tbl


def _build_nc():
    import concourse.bacc as bacc
    import concourse.mybir as mybir
    import concourse.tile as tile

    f16 = mybir.dt.float16
    f32 = mybir.dt.float32
    i16 = mybir.dt.int16
    A = mybir.AluOpType

    nc = bacc.Bacc("TRN2", target_bir_lowering=False, debug=False)
    xp_d = nc.dram_tensor("xp", [BC * H2 * W2], f16, kind="ExternalInput")
    lbx_d = nc.dram_tensor("lbx", [128, NSLOT * Wp], f16, kind="ExternalInput")
    thrx_d = nc.dram_tensor("thrx", [128, NSLOT * Wp], f16, kind="ExternalInput")
    out_d = nc.dram_tensor("out", [BC * H2 * W2 // 2], i16, kind="ExternalOutput")

    with tile.TileContext(nc) as tc:
        with tc.tile_pool(name="const", bufs=1) as cpool, \
             tc.tile_pool(name="bigx", bufs=XBUFS) as xpool, \
             tc.tile_pool(name="big", bufs=BBUFS) as bpool, \
             tc.tile_pool(name="small", bufs=SBUFS) as spool, \
             tc.tile_pool(name="outp", bufs=OBUFS) as opool, \
             tc.tile_pool(name="hoist", bufs=1) as hpool:
            lbt = cpool.tile([128, NSLOT * Wp], f16)
            thrt = cpool.tile([128, NSLOT * Wp], f16)

            def emit_load(j0, P, jpp, xt):
                nel = P * jpp * BLK
                # dense contiguous load: job j -> (partition j%128, slot j//128)
                nc.sync.dma_start(
                    xt[:P, 0:jpp * BLK].rearrange(
                        "p (q f) -> p q f", q=jpp, f=BLK),
                    xp_d[j0 * BLK: j0 * BLK + nel].rearrange(
                        "(q p f) -> p q f", q=jpp, p=P, f=BLK))

            def emit_tables():
                eng = nc.sync if TBL_RING == "sp" else nc.scalar
                eng.dma_start(lbt[:, :], lbx_d[:, :])
                eng.dma_start(thrt[:, :], thrx_d[:, :])

            def do_tile(j0, s0, P, jpp, xt=None, last=False, tail=False, sfx=""):
                """One tile: P partitions x jpp jobs each, jobs j0.., slots s0..
                sfx != "" -> hoisted tile: single dedicated buffers (bufs=1
                pool, allocation sized to jpp) that never gate the main
                pipeline's buffer rotation."""
                # late-tile stores ride the SP ring, which is idle once the
                # load stream finishes; earlier stores use the ACT ring
                st_eng = nc.sync if last else nc.scalar
                QA = jpp if sfx else JPP        # allocation width (jobs)
                pools = (hpool if sfx else xpool, hpool if sfx else bpool,
                         hpool if sfx else spool, hpool if sfx else opool)
                xpoolQ, bpoolQ, spoolQ, opoolQ = pools
                if xt is None:
                    xt = xpoolQ.tile([128, QA * BLK], f16, tag="xt" + sfx)
                    emit_load(j0, P, jpp, xt)
                xv = xt[:, :].rearrange("p (q r c) -> p q r c", q=QA, r=4, c=W2)

                def eng(op):
                    if tail:
                        return nc.vector
                    return nc.gpsimd if ENG[op] == "pool" else nc.vector

                # vertical add (fp16 packed both sides -> 2x DVE mode):
                # v[q, r2, c] = x[q, 2 r2, c] + x[q, 2 r2 + 1, c]
                vt = bpoolQ.tile([128, QA * 2 * W2], f16, tag="vt" + sfx)
                vv = vt[:, :].rearrange("p (q r c) -> p q r c", q=QA, r=2, c=W2)
                eng("v").tensor_tensor(
                    vv[:P, :jpp], xv[:P, :jpp, 0:4:2, :],
                    xv[:P, :jpp, 1:4:2, :], A.add)

                # horizontal conv add: c[q, i, j] = v[q, i, 2j] + v[q, i, 2j+1]
                ct = bpoolQ.tile([128, QA * 2 * HJ], f16, tag="ct" + sfx)
                cv = ct[:, :].rearrange("p (q i j) -> p q i j", q=QA, i=2, j=HJ)
                ncs = CSPLIT if not tail else 0
                jd = HJ - ncs     # columns computed by the main engine
                eng("c").tensor_tensor(
                    cv[:P, :jpp, :, 0:jd], vv[:P, :jpp, :, 0:2 * jd:2],
                    vv[:P, :jpp, :, 1:2 * jd:2], A.add)
                if ncs:
                    nc.gpsimd.tensor_tensor(
                        cv[:P, :jpp, :, jd:HJ], vv[:P, :jpp, :, 2 * jd:W2:2],
                        vv[:P, :jpp, :, 2 * jd + 1:W2:2], A.add)

                def small(tag, dt, n=Wp):
                    tl = spoolQ.tile([128, QA * n], dt, tag=tag + sfx)
                    return tl[:, :].rearrange("p (q j) -> p q j", q=QA)[:P, :jpp]

                # row-pair combine at conv-col resolution (158 wide, 2x)
                s2 = small("s2", f16, HJ)
                eng("s2").tensor_tensor(s2, cv[:P, :jpp, 0, :],
                                        cv[:P, :jpp, 1, :], A.add)
                mn2 = small("mn2", f16, HJ)
                nc.vector.tensor_tensor(mn2, cv[:P, :jpp, 0, :],
                                        cv[:P, :jpp, 1, :], A.min)

                # col-pair combine down to pooled cells (79 wide)
                svt = f16 if SVTM16 else f32
                sv = small("sv", svt)
                eng("sv").tensor_tensor(sv, s2[:, :, 0:HJ:2],
                                        s2[:, :, 1:HJ:2], A.add)
                mv = small("mv", f16)
                nc.vector.tensor_tensor(mv, mn2[:, :, 0:HJ:2],
                                        mn2[:, :, 1:HJ:2], A.min)

                # cond_not = (m >= lb) | (s >= 4 thr m) on the Pool engine;
                # thrt holds clip(4 q1/lb, +-60000) fp16, tm lands in f32
                # (finite).  0/1 -> i16 0x0101/0x0000: the word is two
                # identical output bytes, so the column expansion is free.
                sl = slice(s0 * Wp, (s0 + jpp) * Wp)
                lbv = lbt[:P, sl].rearrange("p (q j) -> p q j", q=jpp)
                thrv = thrt[:P, sl].rearrange("p (q j) -> p q j", q=jpp)
                tm = small("tm", svt)
                eng("tm").tensor_tensor(tm, mv, thrv, A.mult)
                nc1 = small("nc1", f16)
                nc.vector.tensor_tensor(nc1, mv, lbv, A.is_ge)
                nc2 = small("nc2", f16)
                nc.vector.tensor_tensor(nc2, sv, tm, A.is_ge)
                ov = small("ov", f16)
                nc.vector.tensor_tensor(ov, nc1, nc2, A.max)

                # broadcast multiply + row broadcast expand ov[q, j] (0/1) to
                # the output block [q, 4 rows, j, 2 halves] scaled by 257 into
                # i16 0x0101/0x0000 words.  Two ops, each <= 3 free dims (the
                # Activation ISA rejects higher-rank access patterns).
                ob = opoolQ.tile([128, QA * OBLK], i16, tag="ob" + sfx)
                obv = ob[:, :].rearrange("p (q r w) -> p q r w", q=QA, r=4, w=HJ)
                row0 = obv[:P, :jpp, 0, :].rearrange("p q (j k) -> p q j k",
                                                     j=Wp, k=2)
                ovb = ov.unsqueeze(3).broadcast_to([P, jpp, Wp, 2])
                rows = obv[:P, :jpp, 1:4, :]
                r0b = obv[:P, :jpp, 0, :].unsqueeze(2).broadcast_to([P, jpp, 3, HJ])
                e1 = TAILEXP if tail else EXP1
                e2 = TAILEXP if tail else EXP2
                if e1 == "act":
                    nc.scalar.mul(row0, ovb, 257.0)
                elif e1 == "pool":
                    nc.gpsimd.tensor_scalar(row0, ovb, 257.0, None, A.mult)
                else:
                    nc.vector.tensor_scalar(row0, ovb, 257.0, None, A.mult)
                if e2 == "act":
                    nc.scalar.copy(rows, r0b)
                elif e2 == "pool":
                    nc.gpsimd.tensor_copy(rows, r0b)
                else:
                    nc.vector.tensor_copy(rows, r0b)

                # dense contiguous store (1264B per job) on the other ring
                noel = P * jpp * OBLK
                st_eng.dma_start(
                    out_d[j0 * OBLK: j0 * OBLK + noel].rearrange(
                        "(q p f) -> p q f", q=jpp, p=P, f=OBLK),
                    ob[:P, 0:jpp * OBLK].rearrange("p (q f) -> p q f", q=jpp, f=OBLK))

            # job-space offsets per tile
            offs = []
            j0 = 0
            s0 = 0
            for q_n, P in TILES:
                offs.append((j0, s0, P, q_n))
                j0 += q_n * P
                s0 += q_n
            n = len(TILES)
            # emission order: t0, t1, then the last NHOIST job-tiles (their
            # stores are ready early and fill DMA gaps near the end), then
            # the remaining middle tiles in job order
            if ORDER_OVERRIDE is not None:
                order = list(ORDER_OVERRIDE)
            else:
                order = [0, 1] + list(range(n - 1, n - 1 - NHOIST, -1)) + \
                    list(range(2, n - NHOIST))
            assert sorted(order) == list(range(n))
            # pre-emit the first TBL_POS tiles' loads so the table DMAs queue
            # behind them on the ring, then emit tables BEFORE any compute
            # that reads them (no use-before-def)
            pre = {}
            for ei in range(min(TBL_POS, n)):
                ti = order[ei]
                j0, s0, P, q_n = offs[ti]
                hoisted = ti >= n - NHOIST
                sfx = f"_h{ti}" if hoisted else ""
                QA = q_n if sfx else JPP
                xpoolQ = hpool if sfx else xpool
                xt = xpoolQ.tile([128, QA * BLK], f16, tag="xt" + sfx)
                emit_load(j0, P, q_n, xt)
                pre[ti] = xt
            emit_tables()
            for ei, ti in enumerate(order):
                j0, s0, P, q_n = offs[ti]
                hoisted = ti >= n - NHOIST
                do_tile(j0, s0, P, q_n, xt=pre.get(ti),
                        last=ei >= n - 2,
                        tail=ei >= n - NTAIL,
                        sfx=f"_h{ti}" if hoisted else "")
                del j0, s0

    nc.compile()
    return nc


def get_nc():
    if "nc" not in _CACHE:
        _CACHE["nc"] = _build_nc()
    return _CACHE["nc"]


def _check_maps(map_rows, map_cols):
    """The device program hardcodes the clip(4i-1..4i+2) scatter footprint;
    verify the provided maps match it exactly."""
    off = np.arange(4)
    rows = np.clip(4 * np.arange(Hp)[:, None] - 1 + off[None, :], 0, H - 1)
    cols = np.clip(4 * np.arange(Wp)[:, None] - 1 + off[None, :], 0, W - 1)
    exp_rows = np.broadcast_to(rows[:, None, :, None], (Hp, Wp, 4, 4)).reshape(Hp, Wp, 16)
    exp_cols = np.broadcast_to(cols[None, :, None, :], (Hp, Wp, 4, 4)).reshape(Hp, Wp, 16)
    if not (np.asarray(map_rows) == exp_rows).all() or \
       not (np.asarray(map_cols) == exp_cols).all():
        raise ValueError("map_rows/map_cols do not match the expected "
                         "clip(4i-1..4i+2) footprint this kernel hardcodes")


def pad_input(x):
    """[n,1,H,W] (or [n,H,W]) f32 -> flat fp16 [n*H2*W2] with a zero ring."""
    if x.ndim == 4:
        x = x[:, 0]
    xp = np.zeros((x.shape[0], H2, W2), np.float16)
    xp[:, 1:H + 1, 1:W + 1] = x.astype(np.float16)
    return np.ascontiguousarray(xp.reshape(-1))


def kernel(x, lower_bound1, q1, map_rows, map_cols):
    from concourse.bass_utils import run_bass_kernel_spmd

    x = np.asarray(x, dtype=np.float32)
    lb = np.ascontiguousarray(np.asarray(lower_bound1, dtype=np.float32))
    q1 = np.ascontiguousarray(np.asarray(q1, dtype=np.float32))
    _check_maps(map_rows, map_cols)
    assert x.shape == (B, 1, H, W), x.shape

    thr4 = (np.float32(4.0) * (q1 / lb).astype(np.float32)).astype(np.float32)
    lbx = _job_slot_table(lb.astype(np.float16))
    thrx = _job_slot_table(np.clip(thr4, -60000.0, 60000.0).astype(np.float16))

    nc = get_nc()
    in_maps = [
        {"xp": pad_input(x[c * BC:(c + 1) * BC]), "lbx": lbx, "thrx": thrx}
        for c in range(NCORES)
    ]
    res = run_bass_kernel_spmd(nc, in_maps, list(range(NCORES)))
    out = np.concatenate(
        [r["out"].view(np.uint8).reshape(BC, H2, W2)[:, 1:H + 1, 1:W + 1]
         for r in res.results],
        axis=0)
    return np.ascontiguousarray(out.reshape(B, 1, H, W).astype(np.float32))


# revision 59
# speedup vs baseline: 2.3786x; 1.0151x over previous
"""Trainium2 Bass kernel for nn_CNNModel_76312978915482.

Computation (matches the CPU-jax f32 reference within the 2e-2 rel-err gate):
  conv  = 2x2 all-ones conv, stride 2, pad 1 on x [B,1,330,314] -> [B,1,166,158]
  m     = min over each 2x2 conv block            ( == -maxpool(|min(conv,0)|))
  s     = conv sum-pooled 2x2
  cond  = (m < lb) & ((s/4)/m > q1/lb)
  out[r,c] = 1.0 - cond[(r+1)//4 clip, (c+1)//4 clip]   (structured scatter)

This is a memory-regime problem, so the kernel minimizes HBM bytes:
  * x is loaded as fp16 (host converts; the data is N(0,1) so fp16 keeps
    11 mantissa bits). With fp16 intermediates this flips 2332 of 26.5M
    output pixels on the actual dataset -> rel err 1.22e-2 vs the 2e-2
    gate (hardware-verified bit-identical to the numpy model of this
    pipeline).
  * the 0/1 output is produced as int16 words 0x0101/0x0000 (one byte per
    pixel) and stored at 1 byte/pixel; the host reinterprets bytes as f32
    0/1. The division-compare is evaluated as a product compare
    (s >= 4*thr*m); thresholds are fp16 tables clipped to +-60000, so tm
    can only overflow to +-inf with the correct sign (compare-safe, no
    nan is ever produced).
Per core that is 6.71 MB in + 3.36 MB out + 0.85 MB tables ~ 10.9 MB vs
26.9 MB for the all-f32 version: the single shared DMA device (360 GB/s
in the cost model) is busy 30.3 us, and everything else hides under it.

Layout: pure data parallel, batch 256 -> 32 images per core x 8 cores.
The host zero-pads each fp16 image to [332, 316]; a padded image is then
exactly 83 contiguous blocks of 4*316 halves (block I = padded rows
4I..4I+3 = original rows 4I-1..4I+2, one pooled row). Per core that gives
2656 uniform jobs tiled 128 partitions x up to 2 jobs/partition.

Engine split (the real Pool engine only accepts add/mult/scalar/copy ops,
so the min/compare chain must stay on DVE):
  DVE : vertical add (fp16 packed, 2x mode), most of the conv add, min2
        (2x), m, and the three compares
  Pool: sum2, s, 4*thr*m, and a 40-column slice of the conv add
  Act : two broadcast ops expand ov[79] (0/1) -> i16 0x0101 output block
        (x257 pair-broadcast + row broadcast), plus the store DMA ring
  SP  : load DMA ring (threshold tables queue behind the first 4 loads).
The int16 0x0101 trick makes the 4x column expansion a single x257
multiply: the i16 word *is* two identical output bytes, and the 4 rows of
an output block are identical i16 rows (one broadcast copy).
The last two job-tiles are computed FIRST (dedicated buffers) so their
stores fill DMA gaps at the end; the last emitted tiles run their whole
chain on DVE to drain without cross-engine hops.
"""
import numpy as np

B, H, W = 256, 330, 314
Hp, Wp = 83, 79
NCORES = 8
BC = B // NCORES          # images per core (32)
H2, W2 = H + 2, W + 2     # padded image (332, 316)
BLK = 4 * W2              # fp16 elems per job block (1264)
OBLK = BLK // 2           # i16 elems per output job block (632)
HJ = W2 // 2              # conv cols (158)
NJOB = BC * Hp            # jobs per core (2656)
JPP = 4                   # max jobs per partition per tile
# (jobs_per_partition, partitions) per tile; small head tiles fill the
# pipeline quickly, small tail drains it quickly. Sum(jpp*P) == NJOB.
TILES = [(1, 128), (1, 128)] + [(2, 128)] * 8 + [(1, 128), (1, 128)] + [(1, 96)]
assert sum(q * p for q, p in TILES) == NJOB
NSLOT = sum(q for q, _ in TILES)     # lb/thr table slots (21)
XBUFS, BBUFS, SBUFS, OBUFS = 5, 3, 7, 5   # tile-pool depths
NTAIL = 2      # last NTAIL emitted tiles: conds on DVE, short drain chain
TAILEXP = "dve"  # engine for the tail tiles' expansion: act | pool | dve
NHOIST = 2     # compute the last NHOIST job-tiles FIRST (their stores then
               # fill DMA gaps and the job-stream end has no compute drain)
ORDER_OVERRIDE = None   # explicit emission order (list of tile indices)
TBL_RING = "sp"         # ring for the threshold-table DMAs: act | sp
TBL_POS = 4             # table DMAs queue behind this many tile loads
# per-op engine assignment for steady-state tiles (dve | pool).  The real
# Pool engine only accepts add/mult/tensor-scalar/copy TensorTensor forms
# (neuronxcc NCC_IXCG966 rejects min/max/is_ge on Pool), so the min/compare
# chain is pinned to DVE and Pool takes part of the add/mult work.
ENG = {"v": "dve", "c": "dve", "s2": "pool", "sv": "pool", "tm": "pool"}
EXP1, EXP2 = "act", "act"   # engines for the two expansion stages (act|pool|dve)
SVTM16 = True   # sv/tm in fp16: nc2 becomes a 2x packed compare (tm may
                # overflow to +-inf; with thr clipped that is compare-safe)
CSPLIT = 40     # conv-add columns [158-CSPLIT, 158) ride Pool instead of DVE

_CACHE: dict = {}


def _job_slot_table(v, dtype=np.float16):
    """v[Hp, Wp] -> [128, NSLOT*Wp]: per tile t and local slot q, the column
    block on partition p holds v[job % Hp] for job = base_t + q*P_t + p."""
    tbl = np.zeros((128, NSLOT * Wp), dtype)
    base = 0
    s = 0
    for q_n, P in TILES:
        for q in range(q_n):
            jobs = (base + q * P + np.arange(P)) % Hp
            tbl[:P, s * Wp:(s + 1) * Wp] = v[jobs]
            s += 1
        base += q_n * P
    return tbl


def _build_nc():
    import concourse.bacc as bacc
    import concourse.mybir as mybir
    import concourse.tile as tile

    f16 = mybir.dt.float16
    f32 = mybir.dt.float32
    i16 = mybir.dt.int16
    A = mybir.AluOpType

    nc = bacc.Bacc("TRN2", target_bir_lowering=False, debug=False)
    xp_d = nc.dram_tensor("xp", [BC * H2 * W2], f16, kind="ExternalInput")
    # lb and 4*thr slot tables packed side by side -> one table DMA
    tbl_d = nc.dram_tensor("tbl", [128, 2 * NSLOT * Wp], f16, kind="ExternalInput")
    out_d = nc.dram_tensor("out", [BC * H2 * W2 // 2], i16, kind="ExternalOutput")

    with tile.TileContext(nc) as tc:
        with tc.tile_pool(name="const", bufs=1) as cpool, \
             tc.tile_pool(name="bigx", bufs=XBUFS) as xpool, \
             tc.tile_pool(name="big", bufs=BBUFS) as bpool, \
             tc.tile_pool(name="small", bufs=SBUFS) as spool, \
             tc.tile_pool(name="outp", bufs=OBUFS) as opool, \
             tc.tile_pool(name="hoist", bufs=1) as hpool:
            tblt = cpool.tile([128, 2 * NSLOT * Wp], f16)

            def emit_load(j0, P, jpp, xt):
                nel = P * jpp * BLK
                # dense contiguous load: job j -> (partition j%128, slot j//128)
                nc.sync.dma_start(
                    xt[:P, 0:jpp * BLK].rearrange(
                        "p (q f) -> p q f", q=jpp, f=BLK),
                    xp_d[j0 * BLK: j0 * BLK + nel].rearrange(
                        "(q p f) -> p q f", q=jpp, p=P, f=BLK))

            def emit_tables():
                eng = nc.sync if TBL_RING == "sp" else nc.scalar
                eng.dma_start(tblt[:, :], tbl_d[:, :])

            def do_tile(j0, s0, P, jpp, xt=None, last=False, tail=False, sfx=""):
                """One tile: P partitions x jpp jobs each, jobs j0.., slots s0..
                sfx != "" -> hoisted tile: single dedicated buffers (bufs=1
                pool, allocation sized to jpp) that never gate the main
                pipeline's buffer rotation."""
                # late-tile stores ride the SP ring, which is idle once the
                # load stream finishes; earlier stores use the ACT ring
                st_eng = nc.sync if last else nc.scalar
                QA = jpp if sfx else JPP        # allocation width (jobs)
                pools = (hpool if sfx else xpool, hpool if sfx else bpool,
                         hpool if sfx else spool, hpool if sfx else opool)
                xpoolQ, bpoolQ, spoolQ, opoolQ = pools
                if xt is None:
                    xt = xpoolQ.tile([128, QA * BLK], f16, tag="xt" + sfx)
                    emit_load(j0, P, jpp, xt)
                xv = xt[:, :].rearrange("p (q r c) -> p q r c", q=QA, r=4, c=W2)

                def eng(op):
                    if tail:
                        return nc.vector
                    return nc.gpsimd if ENG[op] == "pool" else nc.vector

                # vertical add (fp16 packed both sides -> 2x DVE mode):
                # v[q, r2, c] = x[q, 2 r2, c] + x[q, 2 r2 + 1, c]
                vt = bpoolQ.tile([128, QA * 2 * W2], f16, tag="vt" + sfx)
                vv = vt[:, :].rearrange("p (q r c) -> p q r c", q=QA, r=2, c=W2)
                eng("v").tensor_tensor(
                    vv[:P, :jpp], xv[:P, :jpp, 0:4:2, :],
                    xv[:P, :jpp, 1:4:2, :], A.add)

                # horizontal conv add: c[q, i, j] = v[q, i, 2j] + v[q, i, 2j+1]
                ct = bpoolQ.tile([128, QA * 2 * HJ], f16, tag="ct" + sfx)
                cv = ct[:, :].rearrange("p (q i j) -> p q i j", q=QA, i=2, j=HJ)
                ncs = CSPLIT if not tail else 0
                jd = HJ - ncs     # columns computed by the main engine
                eng("c").tensor_tensor(
                    cv[:P, :jpp, :, 0:jd], vv[:P, :jpp, :, 0:2 * jd:2],
                    vv[:P, :jpp, :, 1:2 * jd:2], A.add)
                if ncs:
                    nc.gpsimd.tensor_tensor(
                        cv[:P, :jpp, :, jd:HJ], vv[:P, :jpp, :, 2 * jd:W2:2],
                        vv[:P, :jpp, :, 2 * jd + 1:W2:2], A.add)

                def small(tag, dt, n=Wp):
                    tl = spoolQ.tile([128, QA * n], dt, tag=tag + sfx)
                    return tl[:, :].rearrange("p (q j) -> p q j", q=QA)[:P, :jpp]

                # row-pair combine at conv-col resolution (158 wide, 2x)
                s2 = small("s2", f16, HJ)
                eng("s2").tensor_tensor(s2, cv[:P, :jpp, 0, :],
                                        cv[:P, :jpp, 1, :], A.add)
                mn2 = small("mn2", f16, HJ)
                nc.vector.tensor_tensor(mn2, cv[:P, :jpp, 0, :],
                                        cv[:P, :jpp, 1, :], A.min)

                # col-pair combine down to pooled cells (79 wide)
                svt = f16 if SVTM16 else f32
                sv = small("sv", svt)
                eng("sv").tensor_tensor(sv, s2[:, :, 0:HJ:2],
                                        s2[:, :, 1:HJ:2], A.add)
                mv = small("mv", f16)
                nc.vector.tensor_tensor(mv, mn2[:, :, 0:HJ:2],
                                        mn2[:, :, 1:HJ:2], A.min)

                # cond_not = (m >= lb) | (s >= 4 thr m) on the Pool engine;
                # thrt holds clip(4 q1/lb, +-60000) fp16, tm lands in f32
                # (finite).  0/1 -> i16 0x0101/0x0000: the word is two
                # identical output bytes, so the column expansion is free.
                lbv = tblt[:P, s0 * Wp:(s0 + jpp) * Wp].rearrange(
                    "p (q j) -> p q j", q=jpp)
                thrv = tblt[:P, (NSLOT + s0) * Wp:(NSLOT + s0 + jpp) * Wp
                            ].rearrange("p (q j) -> p q j", q=jpp)
                tm = small("tm", svt)
                eng("tm").tensor_tensor(tm, mv, thrv, A.mult)
                nc1 = small("nc1", f16)
                nc.vector.tensor_tensor(nc1, mv, lbv, A.is_ge)
                nc2 = small("nc2", f16)
                nc.vector.tensor_tensor(nc2, sv, tm, A.is_ge)
                ov = small("ov", f16)
                nc.vector.tensor_tensor(ov, nc1, nc2, A.max)

                # broadcast multiply + row broadcast expand ov[q, j] (0/1) to
                # the output block [q, 4 rows, j, 2 halves] scaled by 257 into
                # i16 0x0101/0x0000 words.  Two ops, each <= 3 free dims (the
                # Activation ISA rejects higher-rank access patterns).
                ob = opoolQ.tile([128, QA * OBLK], i16, tag="ob" + sfx)
                obv = ob[:, :].rearrange("p (q r w) -> p q r w", q=QA, r=4, w=HJ)
                row0 = obv[:P, :jpp, 0, :].rearrange("p q (j k) -> p q j k",
                                                     j=Wp, k=2)
                ovb = ov.unsqueeze(3).broadcast_to([P, jpp, Wp, 2])
                rows = obv[:P, :jpp, 1:4, :]
                r0b = obv[:P, :jpp, 0, :].unsqueeze(2).broadcast_to([P, jpp, 3, HJ])
                e1 = TAILEXP if tail else EXP1
                e2 = TAILEXP if tail else EXP2
                if e1 == "act":
                    nc.scalar.mul(row0, ovb, 257.0)
                elif e1 == "pool":
                    nc.gpsimd.tensor_scalar(row0, ovb, 257.0, None, A.mult)
                else:
                    nc.vector.tensor_scalar(row0, ovb, 257.0, None, A.mult)
                if e2 == "act":
                    nc.scalar.copy(rows, r0b)
                elif e2 == "pool":
                    nc.gpsimd.tensor_copy(rows, r0b)
                else:
                    nc.vector.tensor_copy(rows, r0b)

                # dense contiguous store (1264B per job) on the other ring
                noel = P * jpp * OBLK
                st_eng.dma_start(
                    out_d[j0 * OBLK: j0 * OBLK + noel].rearrange(
                        "(q p f) -> p q f", q=jpp, p=P, f=OBLK),
                    ob[:P, 0:jpp * OBLK].rearrange("p (q f) -> p q f", q=jpp, f=OBLK))

            # job-space offsets per tile
            offs = []
            j0 = 0
            s0 = 0
            for q_n, P in TILES:
                offs.append((j0, s0, P, q_n))
                j0 += q_n * P
                s0 += q_n
            n = len(TILES)
            # emission order: t0, t1, then the last NHOIST job-tiles (their
            # stores are ready early and fill DMA gaps near the end), then
            # the remaining middle tiles in job order
            if ORDER_OVERRIDE is not None:
                order = list(ORDER_OVERRIDE)
            else:
                order = [0, 1] + list(range(n - 1, n - 1 - NHOIST, -1)) + \
                    list(range(2, n - NHOIST))
            assert sorted(order) == list(range(n))
            # pre-emit the first TBL_POS tiles' loads so the table DMAs queue
            # behind them on the ring, then emit tables BEFORE any compute
            # that reads them (no use-before-def)
            pre = {}
            for ei in range(min(TBL_POS, n)):
                ti = order[ei]
                j0, s0, P, q_n = offs[ti]
                hoisted = ti >= n - NHOIST
                sfx = f"_h{ti}" if hoisted else ""
                QA = q_n if sfx else JPP
                xpoolQ = hpool if sfx else xpool
                xt = xpoolQ.tile([128, QA * BLK], f16, tag="xt" + sfx)
                emit_load(j0, P, q_n, xt)
                pre[ti] = xt
            emit_tables()
            for ei, ti in enumerate(order):
                j0, s0, P, q_n = offs[ti]
                hoisted = ti >= n - NHOIST
                do_tile(j0, s0, P, q_n, xt=pre.get(ti),
                        last=ei >= n - 2,
                        tail=ei >= n - NTAIL,
                        sfx=f"_h{ti}" if hoisted else "")
                del j0, s0

    nc.compile()
    return nc


def get_nc():
    if "nc" not in _CACHE:
        _CACHE["nc"] = _build_nc()
    return _CACHE["nc"]


def _check_maps(map_rows, map_cols):
    """The device program hardcodes the clip(4i-1..4i+2) scatter footprint;
    verify the provided maps match it exactly."""
    off = np.arange(4)
    rows = np.clip(4 * np.arange(Hp)[:, None] - 1 + off[None, :], 0, H - 1)
    cols = np.clip(4 * np.arange(Wp)[:, None] - 1 + off[None, :], 0, W - 1)
    exp_rows = np.broadcast_to(rows[:, None, :, None], (Hp, Wp, 4, 4)).reshape(Hp, Wp, 16)
    exp_cols = np.broadcast_to(cols[None, :, None, :], (Hp, Wp, 4, 4)).reshape(Hp, Wp, 16)
    if not (np.asarray(map_rows) == exp_rows).all() or \
       not (np.asarray(map_cols) == exp_cols).all():
        raise ValueError("map_rows/map_cols do not match the expected "
                         "clip(4i-1..4i+2) footprint this kernel hardcodes")


def pad_input(x):
    """[n,1,H,W] (or [n,H,W]) f32 -> flat fp16 [n*H2*W2] with a zero ring."""
    if x.ndim == 4:
        x = x[:, 0]
    xp = np.zeros((x.shape[0], H2, W2), np.float16)
    xp[:, 1:H + 1, 1:W + 1] = x.astype(np.float16)
    return np.ascontiguousarray(xp.reshape(-1))


def kernel(x, lower_bound1, q1, map_rows, map_cols):
    from concourse.bass_utils import run_bass_kernel_spmd

    x = np.asarray(x, dtype=np.float32)
    lb = np.ascontiguousarray(np.asarray(lower_bound1, dtype=np.float32))
    q1 = np.ascontiguousarray(np.asarray(q1, dtype=np.float32))
    _check_maps(map_rows, map_cols)
    assert x.shape == (B, 1, H, W), x.shape

    thr4 = (np.float32(4.0) * (q1 / lb).astype(np.float32)).astype(np.float32)
    tbl = np.concatenate(
        [_job_slot_table(lb.astype(np.float16)),
         _job_slot_table(np.clip(thr4, -60000.0, 60000.0).astype(np.float16))],
        axis=1)

    nc = get_nc()
    in_maps = [
        {"xp": pad_input(x[c * BC:(c + 1) * BC]), "tbl": tbl}
        for c in range(NCORES)
    ]
    res = run_bass_kernel_spmd(nc, in_maps, list(range(NCORES)))
    out = np.concatenate(
        [r["out"].view(np.uint8).reshape(BC, H2, W2)[:, 1:H + 1, 1:W + 1]
         for r in res.results],
        axis=0)
    return np.ascontiguousarray(out.reshape(B, 1, H, W).astype(np.float32))


# revision 66
# speedup vs baseline: 2.3809x; 1.0010x over previous
"""Trainium2 Bass kernel for nn_CNNModel_76312978915482.

Computation (matches the CPU-jax f32 reference within the 2e-2 rel-err gate):
  conv  = 2x2 all-ones conv, stride 2, pad 1 on x [B,1,330,314] -> [B,1,166,158]
  m     = min over each 2x2 conv block            ( == -maxpool(|min(conv,0)|))
  s     = conv sum-pooled 2x2
  cond  = (m < lb) & ((s/4)/m > q1/lb)
  out[r,c] = 1.0 - cond[(r+1)//4 clip, (c+1)//4 clip]   (structured scatter)

This is a memory-regime problem, so the kernel minimizes HBM bytes:
  * x is loaded as fp16 (host converts; the data is N(0,1) so fp16 keeps
    11 mantissa bits). With fp16 intermediates this flips 2332 of 26.5M
    output pixels on the actual dataset -> rel err 1.22e-2 vs the 2e-2
    gate (hardware-verified bit-identical to the numpy model of this
    pipeline).
  * the 0/1 output is produced as int16 words 0x0101/0x0000 (one byte per
    pixel) and stored at 1 byte/pixel; the host reinterprets bytes as f32
    0/1. The division-compare is evaluated as a product compare
    (s >= 4*thr*m); thresholds are fp16 tables clipped to +-60000, so tm
    can only overflow to +-inf with the correct sign (compare-safe, no
    nan is ever produced).
Per core that is 6.71 MB in + 3.36 MB out + 0.85 MB tables ~ 10.9 MB vs
26.9 MB for the all-f32 version: the single shared DMA device (360 GB/s
in the cost model) is busy 30.3 us, and everything else hides under it.

Layout: pure data parallel, batch 256 -> 32 images per core x 8 cores.
The host zero-pads each fp16 image to [332, 316]; a padded image is then
exactly 83 contiguous blocks of 4*316 halves (block I = padded rows
4I..4I+3 = original rows 4I-1..4I+2, one pooled row). Per core that gives
2656 uniform jobs tiled 128 partitions x up to 2 jobs/partition.

Engine split (the real Pool engine only accepts add/mult/scalar/copy ops,
so the min/compare chain must stay on DVE):
  DVE : vertical add (fp16 packed, 2x mode), most of the conv add, min2
        (2x), m, and the three compares
  Pool: sum2, s, 4*thr*m, and a 40-column slice of the conv add
  Act : two broadcast ops expand ov[79] (0/1) -> i16 0x0101 output block
        (x257 pair-broadcast + row broadcast), plus the store DMA ring
  SP  : load DMA ring (the single packed lb/thr table DMA queues behind
        the first 4 tile loads).
The int16 0x0101 trick makes the 4x column expansion a single x257
multiply: the i16 word *is* two identical output bytes, and the 4 rows of
an output block are identical i16 rows (one broadcast copy).
The last two job-tiles are computed FIRST (dedicated buffers) so their
stores fill DMA gaps at the end; the last emitted tiles run their whole
chain on DVE to drain without cross-engine hops.
"""
import numpy as np

B, H, W = 256, 330, 314
Hp, Wp = 83, 79
NCORES = 8
BC = B // NCORES          # images per core (32)
H2, W2 = H + 2, W + 2     # padded image (332, 316)
BLK = 4 * W2              # fp16 elems per job block (1264)
OBLK = BLK // 2           # i16 elems per output job block (632)
HJ = W2 // 2              # conv cols (158)
NJOB = BC * Hp            # jobs per core (2656)
JPP = 4                   # max jobs per partition per tile
# (jobs_per_partition, partitions) per tile; small head tiles fill the
# pipeline quickly, small tail drains it quickly. Sum(jpp*P) == NJOB.
TILES = [(1, 128), (1, 128)] + [(2, 128)] * 8 + [(1, 128), (1, 128)] + [(1, 96)]
assert sum(q * p for q, p in TILES) == NJOB
NSLOT = sum(q for q, _ in TILES)     # lb/thr table slots (21)
XBUFS, BBUFS, SBUFS, OBUFS = 5, 3, 7, 5   # tile-pool depths
NTAIL = 2      # last NTAIL emitted tiles: conds on DVE, short drain chain
TAILEXP = "dve"  # engine for the tail tiles' expansion: act | pool | dve
NHOIST = 2     # compute the last NHOIST job-tiles FIRST (their stores then
               # fill DMA gaps and the job-stream end has no compute drain)
ORDER_OVERRIDE = None   # explicit emission order (list of tile indices)
TBL_RING = "sp"         # ring for the threshold-table DMAs: act | sp
TBL_POS = 4             # table DMAs queue behind this many tile loads
# per-op engine assignment for steady-state tiles (dve | pool).  The real
# Pool engine only accepts add/mult/tensor-scalar/copy TensorTensor forms
# (neuronxcc NCC_IXCG966 rejects min/max/is_ge on Pool), so the min/compare
# chain is pinned to DVE and Pool takes part of the add/mult work.
ENG = {"v": "dve", "c": "dve", "s2": "pool", "sv": "pool", "tm": "pool"}
EXP1, EXP2 = "act", "act"   # engines for the two expansion stages (act|pool|dve)
SVTM16 = True   # sv/tm in fp16: nc2 becomes a 2x packed compare (tm may
                # overflow to +-inf; with thr clipped that is compare-safe)
CSPLIT = 32     # conv-add columns [158-CSPLIT, 158) ride Pool instead of DVE
SPLIT_IO = True  # jpp-4 tiles: halve the DMA/expansion granularity (two
                 # loads, two expansions, two stores) while compute ops span
                 # all 4 jobs in one instruction (fewer per-inst inits)

_CACHE: dict = {}


def _job_slot_table(v, dtype=np.float16):
    """v[Hp, Wp] -> [128, NSLOT*Wp]: per tile t and local slot q, the column
    block on partition p holds v[job % Hp] for job = base_t + q*P_t + p."""
    tbl = np.zeros((128, NSLOT * Wp), dtype)
    base = 0
    s = 0
    for q_n, P in TILES:
        for q in range(q_n):
            jobs = (base + q * P + np.arange(P)) % Hp
            tbl[:P, s * Wp:(s + 1) * Wp] = v[jobs]
            s += 1
        base += q_n * P
    return tbl


def _build_nc():
    import concourse.bacc as bacc
    import concourse.mybir as mybir
    import concourse.tile as tile

    f16 = mybir.dt.float16
    f32 = mybir.dt.float32
    i16 = mybir.dt.int16
    A = mybir.AluOpType

    nc = bacc.Bacc("TRN2", target_bir_lowering=False, debug=False)
    xp_d = nc.dram_tensor("xp", [BC * H2 * W2], f16, kind="ExternalInput")
    # lb and 4*thr slot tables packed side by side -> one table DMA
    tbl_d = nc.dram_tensor("tbl", [128, 2 * NSLOT * Wp], f16, kind="ExternalInput")
    out_d = nc.dram_tensor("out", [BC * H2 * W2 // 2], i16, kind="ExternalOutput")

    with tile.TileContext(nc) as tc:
        with tc.tile_pool(name="const", bufs=1) as cpool, \
             tc.tile_pool(name="bigx", bufs=XBUFS) as xpool, \
             tc.tile_pool(name="big", bufs=BBUFS) as bpool, \
             tc.tile_pool(name="small", bufs=SBUFS) as spool, \
             tc.tile_pool(name="outp", bufs=OBUFS) as opool, \
             tc.tile_pool(name="hoist", bufs=1) as hpool:
            tblt = cpool.tile([128, 2 * NSLOT * Wp], f16)

            def emit_load(j0, P, jpp, xt, qoff=0):
                nel = P * jpp * BLK
                # dense contiguous load: job j -> (partition j%128, slot j//128)
                nc.sync.dma_start(
                    xt[:P, qoff * BLK:(qoff + jpp) * BLK].rearrange(
                        "p (q f) -> p q f", q=jpp, f=BLK),
                    xp_d[j0 * BLK: j0 * BLK + nel].rearrange(
                        "(q p f) -> p q f", q=jpp, p=P, f=BLK))

            def emit_tables():
                eng = nc.sync if TBL_RING == "sp" else nc.scalar
                eng.dma_start(tblt[:, :], tbl_d[:, :])

            def do_tile(j0, s0, P, jpp, xt=None, last=False, tail=False, sfx=""):
                """One tile: P partitions x jpp jobs each, jobs j0.., slots s0..
                sfx != "" -> hoisted tile: single dedicated buffers (bufs=1
                pool, allocation sized to jpp) that never gate the main
                pipeline's buffer rotation."""
                # late-tile stores ride the SP ring, which is idle once the
                # load stream finishes; earlier stores use the ACT ring
                st_eng = nc.sync if last else nc.scalar
                QA = jpp if sfx else JPP        # allocation width (jobs)
                pools = (hpool if sfx else xpool, hpool if sfx else bpool,
                         hpool if sfx else spool, hpool if sfx else opool)
                xpoolQ, bpoolQ, spoolQ, opoolQ = pools
                segs = [(0, 2), (2, 4)] if (jpp == 4 and SPLIT_IO) else [(0, jpp)]
                if xt is None:
                    xt = xpoolQ.tile([128, QA * BLK], f16, tag="xt" + sfx)
                    for qlo, qhi in segs:
                        emit_load(j0 + qlo * P, P, qhi - qlo, xt, qoff=qlo)
                xv = xt[:, :].rearrange("p (q r c) -> p q r c", q=QA, r=4, c=W2)

                def eng(op):
                    if tail:
                        return nc.vector
                    return nc.gpsimd if ENG[op] == "pool" else nc.vector

                # vertical add (fp16 packed both sides -> 2x DVE mode):
                # v[q, r2, c] = x[q, 2 r2, c] + x[q, 2 r2 + 1, c]
                vt = bpoolQ.tile([128, QA * 2 * W2], f16, tag="vt" + sfx)
                vv = vt[:, :].rearrange("p (q r c) -> p q r c", q=QA, r=2, c=W2)
                eng("v").tensor_tensor(
                    vv[:P, :jpp], xv[:P, :jpp, 0:4:2, :],
                    xv[:P, :jpp, 1:4:2, :], A.add)

                # horizontal conv add: c[q, i, j] = v[q, i, 2j] + v[q, i, 2j+1]
                ct = bpoolQ.tile([128, QA * 2 * HJ], f16, tag="ct" + sfx)
                cv = ct[:, :].rearrange("p (q i j) -> p q i j", q=QA, i=2, j=HJ)
                ncs = CSPLIT if not tail else 0
                jd = HJ - ncs     # columns computed by the main engine
                eng("c").tensor_tensor(
                    cv[:P, :jpp, :, 0:jd], vv[:P, :jpp, :, 0:2 * jd:2],
                    vv[:P, :jpp, :, 1:2 * jd:2], A.add)
                if ncs:
                    nc.gpsimd.tensor_tensor(
                        cv[:P, :jpp, :, jd:HJ], vv[:P, :jpp, :, 2 * jd:W2:2],
                        vv[:P, :jpp, :, 2 * jd + 1:W2:2], A.add)

                def small(tag, dt, n=Wp):
                    tl = spoolQ.tile([128, QA * n], dt, tag=tag + sfx)
                    return tl[:, :].rearrange("p (q j) -> p q j", q=QA)[:P, :jpp]

                # row-pair combine at conv-col resolution (158 wide, 2x)
                s2 = small("s2", f16, HJ)
                eng("s2").tensor_tensor(s2, cv[:P, :jpp, 0, :],
                                        cv[:P, :jpp, 1, :], A.add)
                mn2 = small("mn2", f16, HJ)
                nc.vector.tensor_tensor(mn2, cv[:P, :jpp, 0, :],
                                        cv[:P, :jpp, 1, :], A.min)

                # col-pair combine down to pooled cells (79 wide)
                svt = f16 if SVTM16 else f32
                sv = small("sv", svt)
                eng("sv").tensor_tensor(sv, s2[:, :, 0:HJ:2],
                                        s2[:, :, 1:HJ:2], A.add)
                mv = small("mv", f16)
                nc.vector.tensor_tensor(mv, mn2[:, :, 0:HJ:2],
                                        mn2[:, :, 1:HJ:2], A.min)

                # cond_not = (m >= lb) | (s >= 4 thr m) on the Pool engine;
                # thrt holds clip(4 q1/lb, +-60000) fp16, tm lands in f32
                # (finite).  0/1 -> i16 0x0101/0x0000: the word is two
                # identical output bytes, so the column expansion is free.
                lbv = tblt[:P, s0 * Wp:(s0 + jpp) * Wp].rearrange(
                    "p (q j) -> p q j", q=jpp)
                thrv = tblt[:P, (NSLOT + s0) * Wp:(NSLOT + s0 + jpp) * Wp
                            ].rearrange("p (q j) -> p q j", q=jpp)
                tm = small("tm", svt)
                eng("tm").tensor_tensor(tm, mv, thrv, A.mult)
                nc1 = small("nc1", f16)
                nc.vector.tensor_tensor(nc1, mv, lbv, A.is_ge)
                nc2 = small("nc2", f16)
                nc.vector.tensor_tensor(nc2, sv, tm, A.is_ge)
                ov = small("ov", f16)
                nc.vector.tensor_tensor(ov, nc1, nc2, A.max)

                # broadcast multiply + row broadcast expand ov[q, j] (0/1) to
                # the output block [q, 4 rows, j, 2 halves] scaled by 257 into
                # i16 0x0101/0x0000 words.  Two ops, each <= 3 free dims (the
                # Activation ISA rejects higher-rank access patterns).
                ob = opoolQ.tile([128, QA * OBLK], i16, tag="ob" + sfx)
                obv = ob[:, :].rearrange("p (q r w) -> p q r w", q=QA, r=4, w=HJ)
                e1 = TAILEXP if tail else EXP1
                e2 = TAILEXP if tail else EXP2
                for qlo, qhi in segs:
                    qn = qhi - qlo
                    row0 = obv[:P, qlo:qhi, 0, :].rearrange(
                        "p q (j k) -> p q j k", j=Wp, k=2)
                    ovb = ov[:, qlo:qhi].unsqueeze(3).broadcast_to([P, qn, Wp, 2])
                    rows = obv[:P, qlo:qhi, 1:4, :]
                    r0b = obv[:P, qlo:qhi, 0, :].unsqueeze(2).broadcast_to(
                        [P, qn, 3, HJ])
                    if e1 == "act":
                        nc.scalar.mul(row0, ovb, 257.0)
                    elif e1 == "pool":
                        nc.gpsimd.tensor_scalar(row0, ovb, 257.0, None, A.mult)
                    else:
                        nc.vector.tensor_scalar(row0, ovb, 257.0, None, A.mult)
                    if e2 == "act":
                        nc.scalar.copy(rows, r0b)
                    elif e2 == "pool":
                        nc.gpsimd.tensor_copy(rows, r0b)
                    else:
                        nc.vector.tensor_copy(rows, r0b)

                    # dense contiguous store (1264B per job) on the other ring
                    st_eng.dma_start(
                        out_d[(j0 + qlo * P) * OBLK:
                              (j0 + qhi * P) * OBLK].rearrange(
                            "(q p f) -> p q f", q=qn, p=P, f=OBLK),
                        ob[:P, qlo * OBLK:qhi * OBLK].rearrange(
                            "p (q f) -> p q f", q=qn, f=OBLK))

            # job-space offsets per tile
            offs = []
            j0 = 0
            s0 = 0
            for q_n, P in TILES:
                offs.append((j0, s0, P, q_n))
                j0 += q_n * P
                s0 += q_n
            n = len(TILES)
            # emission order: t0, t1, then the last NHOIST job-tiles (their
            # stores are ready early and fill DMA gaps near the end), then
            # the remaining middle tiles in job order
            if ORDER_OVERRIDE is not None:
                order = list(ORDER_OVERRIDE)
            else:
                order = [0, 1] + list(range(n - 1, n - 1 - NHOIST, -1)) + \
                    list(range(2, n - NHOIST))
            assert sorted(order) == list(range(n))
            # pre-emit the first TBL_POS tiles' loads so the table DMAs queue
            # behind them on the ring, then emit tables BEFORE any compute
            # that reads them (no use-before-def)
            pre = {}
            for ei in range(min(TBL_POS, n)):
                ti = order[ei]
                j0, s0, P, q_n = offs[ti]
                hoisted = ti >= n - NHOIST
                sfx = f"_h{ti}" if hoisted else ""
                QA = q_n if sfx else JPP
                xpoolQ = hpool if sfx else xpool
                xt = xpoolQ.tile([128, QA * BLK], f16, tag="xt" + sfx)
                for qlo, qhi in ([(0, 2), (2, 4)]
                                 if (q_n == 4 and SPLIT_IO) else [(0, q_n)]):
                    emit_load(j0 + qlo * P, P, qhi - qlo, xt, qoff=qlo)
                pre[ti] = xt
            emit_tables()
            for ei, ti in enumerate(order):
                j0, s0, P, q_n = offs[ti]
                hoisted = ti >= n - NHOIST
                do_tile(j0, s0, P, q_n, xt=pre.get(ti),
                        last=ei >= n - 2,
                        tail=ei >= n - NTAIL,
                        sfx=f"_h{ti}" if hoisted else "")
                del j0, s0

    nc.compile()
    return nc


def get_nc():
    if "nc" not in _CACHE:
        _CACHE["nc"] = _build_nc()
    return _CACHE["nc"]


def _check_maps(map_rows, map_cols):
    """The device program hardcodes the clip(4i-1..4i+2) scatter footprint;
    verify the provided maps match it exactly."""
    off = np.arange(4)
    rows = np.clip(4 * np.arange(Hp)[:, None] - 1 + off[None, :], 0, H - 1)
    cols = np.clip(4 * np.arange(Wp)[:, None] - 1 + off[None, :], 0, W - 1)
    exp_rows = np.broadcast_to(rows[:, None, :, None], (Hp, Wp, 4, 4)).reshape(Hp, Wp, 16)
    exp_cols = np.broadcast_to(cols[None, :, None, :], (Hp, Wp, 4, 4)).reshape(Hp, Wp, 16)
    if not (np.asarray(map_rows) == exp_rows).all() or \
       not (np.asarray(map_cols) == exp_cols).all():
        raise ValueError("map_rows/map_cols do not match the expected "
                         "clip(4i-1..4i+2) footprint this kernel hardcodes")


def pad_input(x):
    """[n,1,H,W] (or [n,H,W]) f32 -> flat fp16 [n*H2*W2] with a zero ring."""
    if x.ndim == 4:
        x = x[:, 0]
    xp = np.zeros((x.shape[0], H2, W2), np.float16)
    xp[:, 1:H + 1, 1:W + 1] = x.astype(np.float16)
    return np.ascontiguousarray(xp.reshape(-1))


def kernel(x, lower_bound1, q1, map_rows, map_cols):
    from concourse.bass_utils import run_bass_kernel_spmd

    x = np.asarray(x, dtype=np.float32)
    lb = np.ascontiguousarray(np.asarray(lower_bound1, dtype=np.float32))
    q1 = np.ascontiguousarray(np.asarray(q1, dtype=np.float32))
    _check_maps(map_rows, map_cols)
    assert x.shape == (B, 1, H, W), x.shape

    thr4 = (np.float32(4.0) * (q1 / lb).astype(np.float32)).astype(np.float32)
    tbl = np.concatenate(
        [_job_slot_table(lb.astype(np.float16)),
         _job_slot_table(np.clip(thr4, -60000.0, 60000.0).astype(np.float16))],
        axis=1)

    nc = get_nc()
    in_maps = [
        {"xp": pad_input(x[c * BC:(c + 1) * BC]), "tbl": tbl}
        for c in range(NCORES)
    ]
    res = run_bass_kernel_spmd(nc, in_maps, list(range(NCORES)))
    out = np.concatenate(
        [r["out"].view(np.uint8).reshape(BC, H2, W2)[:, 1:H + 1, 1:W + 1]
         for r in res.results],
        axis=0)
    return np.ascontiguousarray(out.reshape(B, 1, H, W).astype(np.float32))
